# revision 2
# baseline (speedup 1.0000x reference)
"""Trainium2 Bass kernel for policy-masked attention (sparse_attention), v3.

Shapes (hardcoded): x [4,1024,768], decision [4,768,2], qkv_w [2304,768],
qkv_b [2304], proj_w [768,768], proj_b [768], search_feat_len=768.

Sharding: 8 cores = 4 batches x 2 head-groups (6 heads each). Each core
computes its batch's q/k/v for its heads, the policy-masked softmax
(one-hot policy folded into the score matmul as 2 extra contraction rows
of -BIG * indicator outer products), attn @ v with a fused ones-column
producing the softmax denominator, and a partial output projection.
Host sums the two head-group partials per batch.

v3 design:
- all PE operands bf16 (same PE rate as fp32r, half DMA/SBUF).
- host sorts tokens [template | group2 | group1] per batch, so the two
  masked blocks (template-q x group1-k and group1-q x template-k) become
  contiguous; score/exp/AV skip the conservatively-safe common region
  (bounds shared across cores: q1 = tpl + max_b g2). The rank-2 -BIG mask
  rows still handle all boundary tiles exactly.
- interleaved emission: QK-gen j-groups and the first four heads' score
  streams alternate, so the scalar engine's exp stream starts ~6us in and
  runs gapless; AV matmuls for those heads run later against buffered e
  tiles. V-gen sits between; proj tt0/tt1 matmuls overlap the last
  Z-chain.
- softmax 1/Z: DVE reciprocal of the fused Z row, PE ones-matmul
  broadcast, one PSUM->SBUF stage (single-PSUM-operand rule), DVE muls.
- AV accumulation emits the full-width m=2 tile first so every PSUM
  element is has_written-initialized regardless of clear granularity.
"""
import numpy as np
import ml_dtypes

import concourse.bass as bass
import concourse.tile as tile
from concourse import bacc, mybir
from concourse.bass_utils import run_bass_kernel_spmd

F32 = mybir.dt.float32
BF16 = mybir.dt.bfloat16
AF = mybir.ActivationFunctionType
ALU = mybir.AluOpType
BFNP = ml_dtypes.bfloat16

B, N, C = 4, 1024, 768
H = 12
HD = 64
HPC = 6              # heads per core
KT = C // 128        # 6 contraction tiles
NT = N // 512        # 2 moving slices
MT = N // 128        # 8 key tiles
CO = C // 128        # 6 output-column tiles
SCALE = HD ** -0.5
BIG = 32768.0
N_CORES = 8


def _score_ranges(m, q1, m1s):
    """Query-column ranges to compute for key-tile m (512-col chunks)."""
    if m < 2 and q1 < N:          # template keys: skip group1 queries
        return [(0, 512), (512, q1)] if q1 > 512 else [(0, q1)]
    if m >= m1s:                  # group1 keys: skip template queries
        return [(256, 512), (512, 1024)]
    return [(0, 512), (512, 1024)]


def _exp_range(m, q1, m1s):
    if m < 2 and q1 < N:
        return (0, q1)
    if m >= m1s:
        return (256, N)
    return (0, N)


def _body(nc, tc, t, with_vbias, q1, m1s):
    import contextlib
    with contextlib.ExitStack() as ctx:
        consts = ctx.enter_context(tc.tile_pool(name="consts", bufs=1))
        headp = ctx.enter_context(tc.tile_pool(name="headp", bufs=1))

        xT_sb = consts.tile([128, KT, N], BF16)
        wqkT_sb = consts.tile([128, KT, C], BF16)
        wvT_sb = consts.tile([128, KT, HPC * HD], BF16)
        vbias_sb = consts.tile([1, HPC * HD], BF16)
        ones1_sb = consts.tile([1, 128], BF16)
        ones64_sb = consts.tile([1, 64], BF16)
        qkb_sb = consts.tile([128, KT], F32)
        projT_sb = consts.tile([128, 3, C], BF16)
        pbias_sb = consts.tile([128, CO], F32)

        # Per-head q/k tiles [66, N]: rows 0-63 head data, rows 64-65 the
        # rank-2 log-mask factors (k side: -BIG*p0,-BIG*p1; q side: p1,p0).
        qh = [headp.tile([66, N], BF16, name=f"qh{h}", tag=f"qh{h}")
              for h in range(HPC)]
        kh = [headp.tile([66, N], BF16, name=f"kh{h}", tag=f"kh{h}")
              for h in range(HPC)]

        # Every dma_start costs ~1.2us of SEQ time on the issuing engine, so
        # batch the inputs into few large DMAs and keep the scalar (ACT)
        # queue nearly empty -- its sequencer must be free for the exp
        # stream. wqkT is stored j-interleaved ([j0|j3|j1|j4|j2|j5]) so each
        # group's 256 weight columns are one strided DMA; x lands in three
        # ascending pieces so group 0's kt-outer matmuls start early.
        nc.scalar.dma_start(out=wqkT_sb[:, :, 0:256],
                            in_=t["wqkT"].ap()[:, :, 0:256])
        nc.scalar.dma_start(out=qh[0][64:66, :], in_=t["mq"].ap())
        nc.scalar.dma_start(out=qkb_sb, in_=t["qkb"].ap())
        if with_vbias:
            nc.scalar.dma_start(out=vbias_sb, in_=t["vbias"].ap())
        nc.sync.dma_start(out=xT_sb[:, 0, :], in_=t["xT"].ap()[:, 0, :])
        nc.sync.dma_start(out=kh[0][64:66, :], in_=t["mk"].ap())
        nc.sync.dma_start(out=xT_sb[:, 1:3, :], in_=t["xT"].ap()[:, 1:3, :])
        nc.sync.dma_start(out=xT_sb[:, 3:6, :], in_=t["xT"].ap()[:, 3:6, :])
        nc.sync.dma_start(out=wqkT_sb[:, :, 256:768],
                          in_=t["wqkT"].ap()[:, :, 256:768])
        nc.sync.dma_start(out=wvT_sb, in_=t["wvT"].ap())
        for h in range(1, HPC):
            nc.sync.dma_start(out=qh[h][64:66, :], in_=t["mq"].ap())
            nc.sync.dma_start(out=kh[h][64:66, :], in_=t["mk"].ap())
        nc.sync.dma_start(out=pbias_sb, in_=t["pbias"].ap())
        nc.sync.dma_start(out=projT_sb, in_=t["projT"].ap())
        nc.vector.memset(ones1_sb, 1.0)
        nc.vector.memset(ones64_sb, 1.0)

        # V in token-major layout with a fused ones column: [128, MT, 6*65]
        V_sb = consts.tile([128, MT, HPC * 65], BF16)
        vv = V_sb.rearrange("p m (h e) -> p m h e", h=HPC)
        nc.vector.memset(vv[:, :, :, 64:65], 1.0)

        # Pre-load the Exp activation table off the critical path.
        warm = consts.tile([1, 1], F32)
        nc.scalar.activation(warm, qkb_sb[0:1, 0:1], AF.Exp)


        abp = ctx.enter_context(tc.tile_pool(name="abp", bufs=1))
        Ab = abp.tile([128, 3, N], BF16)

        stp = ctx.enter_context(tc.tile_pool(name="stp", bufs=2, space="PSUM"))

        # Warm up the PE p-state/HAM during the initial DMA window: ~4.5us
        # of dummy matmuls on memset constants so the real QK-gen matmuls
        # run at full clock. Serialized WAW on one PSUM tile keeps them
        # back-to-back; one DVE read is the ring consumer.
        wrhs = consts.tile([1, 512], BF16)
        nc.vector.memset(wrhs, 0.5)
        wps = stp.tile([64, 512], F32, tag="st", name="wps")
        for _ in range(9):
            nc.tensor.matmul(wps, lhsT=ones64_sb, rhs=wrhs,
                             start=True, stop=True)
        wsink = consts.tile([1, 1], F32)
        nc.vector.tensor_copy(wsink, wps[0:1, 0:1])

        ep = ctx.enter_context(tc.tile_pool(name="ep", bufs=48))
        zp = ctx.enter_context(tc.tile_pool(name="zp", bufs=4))

        ups, zrec, ust, etiles = {}, {}, {}, {}

        # ---- emission helpers ------------------------------------------
        def emit_grp(ps1, g, act_copies=False):
            """QK-gen for j-group {g, g+3} -> heads 2g, 2g+1. kt-outer and
            n-inner: all four [128,512] PSUM tiles accumulate together, so
            the matmuls pace with the x/w DMA chunk stream. wqkT column
            block 2g holds the q rows, 2g+1 the k rows (j-interleaved)."""
            for i, (p, n) in enumerate(
                    (p, n) for p in (2 * g, 2 * g + 1) for n in range(NT)):
                ps = ps1.tile([128, 512], F32, bufs=1,
                              tag=f"g{'qk'[p % 2]}{n}", name=f"g{g}_{p}_{n}")
                for kt in range(KT):
                    nc.tensor.matmul(
                        ps,
                        lhsT=wqkT_sb[:, kt, p * 128:(p + 1) * 128],
                        rhs=xT_sb[:, kt, n * 512:(n + 1) * 512],
                        start=(kt == 0), stop=(kt == KT - 1))
                tiles = qh if p % 2 == 0 else kh
                sl = slice(n * 512, (n + 1) * 512)
                for half in range(2):
                    h = 2 * g + half
                    dst = tiles[h][0:64, sl]
                    srcap = ps[half * 64:(half + 1) * 64, :]
                    bias = qkb_sb[half * 64:(half + 1) * 64, p:p + 1]
                    if act_copies and (i + half) % 2 == 1:
                        nc.scalar.activation(dst, srcap, AF.Identity,
                                             bias=bias, scale=1.0)
                    else:
                        nc.vector.tensor_scalar(out=dst, in0=srcap,
                                                scalar1=bias, scalar2=None,
                                                op0=ALU.add)

        def emit_vgen(ps1):
            for m in range(MT):
                psv = ps1.tile([128, HPC * HD], F32,
                               tag=f"gq{m % 2}", bufs=1)
                for kt in range(KT):
                    nc.tensor.matmul(psv,
                                     lhsT=xT_sb[:, kt, m * 128:(m + 1) * 128],
                                     rhs=wvT_sb[:, kt, :],
                                     start=(kt == 0),
                                     stop=(not with_vbias and kt == KT - 1))
                if with_vbias:
                    nc.tensor.matmul(psv, lhsT=ones1_sb, rhs=vbias_sb,
                                     start=False, stop=True)
                nc.vector.tensor_copy(vv[:, m, :, 0:64],
                                      psv.rearrange("p (h d) -> p h d", h=HPC))

        def emit_scores(h, weave=()):
            """All 8 key-tiles of head h: score matmuls + exp. Up to two
            backlog emitters (AV matmuls / Z-chains) are woven in after each
            key-tile so the PE fills the exp-paced slack."""
            wq = list(weave)
            for m in range(MT):
                st = stp.tile([128, N], F32, tag="st", name=f"st{h}_{m}")
                for a, b in _score_ranges(m, q1, m1s):
                    nc.tensor.matmul(st[:, a:b],
                                     lhsT=kh[h][:, m * 128:(m + 1) * 128],
                                     rhs=qh[h][:, a:b],
                                     start=True, stop=True,
                                     skip_group_check=True)
                e = ep.tile([128, N], BF16, tag="e", name=f"e{h}_{m}")
                ea, eb = _exp_range(m, q1, m1s)
                nc.scalar.activation(e[:, ea:eb], st[:, ea:eb], AF.Exp)
                etiles[(h, m)] = e
                for _ in range(2):
                    if wq:
                        wq.pop(0)()
            for f in wq:
                f()

        def av_chunks(h, up):
            """Per-key-tile emitters for head h's AV accumulation. m=2
            (always full-width) goes first so every PSUM element is
            initialized by a start-group matmul; the partial-width tiles
            then accumulate per-element. The final emitter computes 1/Z
            (EPS is negligible: Z >= exp(s_ii); bf16 is a 0.4% common-mode
            scale on one head-query)."""
            order = [2] + [m for m in range(MT) if m != 2]

            def mk(i, m):
                def f():
                    if i == 0:
                        ups[h] = up.tile([65, N], F32, name=f"u{h}", tag="u")
                    e = etiles.pop((h, m))
                    rs = _score_ranges(m, q1, m1s)
                    for k, (a, b) in enumerate(rs):
                        nc.tensor.matmul(
                            ups[h][:, a:b],
                            lhsT=V_sb[:, m, h * 65:(h + 1) * 65],
                            rhs=e[:, a:b],
                            start=(i == 0),
                            stop=(i == len(order) - 1 and k == len(rs) - 1),
                            skip_group_check=True)
                return f

            def zf():
                # recip (DVE) and the eager U->SBUF copy (ACT for the last
                # pair, DVE otherwise) are the only ups readers, so the
                # ups ring slot recycles ~2.4us sooner than a PSUM-side
                # normalize would allow.
                zrec[h] = zp.tile([1, N], BF16, tag="z", name=f"z{h}")
                with nc.allow_low_precision(reason="1/Z common-mode scale"):
                    nc.vector.reciprocal(zrec[h], ups[h][64:65, :])
                ust[h] = zp.tile([64, N], BF16, tag="ust", name=f"ust{h}")
                if h >= 4:
                    nc.scalar.activation(ust[h], ups[h][0:64, :], AF.Copy)
                else:
                    nc.vector.tensor_copy(ust[h], ups[h][0:64, :])

            return [mk(i, m) for i, m in enumerate(order)] + [zf]

        def emit_av(h, up):
            for f in av_chunks(h, up):
                f()

        def emit_zchain(h, zbs_eng=None):
            """Per-head: broadcast 1/Z across partitions with a ones-matmul
            into an stp-ring slot, then normalize this head's half of its
            pair from the eagerly-staged SBUF copy: Ab = Ust * (1/Z)."""
            tt, base = h // 2, 64 * (h % 2)
            zbh = stp.tile([64, N], F32, tag="st", name=f"zbh{h}")
            for n in range(NT):
                sl = slice(n * 512, (n + 1) * 512)
                nc.tensor.matmul(zbh[:, sl], lhsT=ones64_sb,
                                 rhs=zrec[h][:, sl], start=True, stop=True)
            if h == HPC - 1:
                # split so the first proj tt2 matmuls start half a mul early
                for n in range(NT):
                    sl = slice(n * 512, (n + 1) * 512)
                    nc.vector.tensor_mul(Ab[base:base + 64, tt, sl],
                                         ust[h][:, sl], zbh[:, sl])
            else:
                nc.vector.tensor_mul(Ab[base:base + 64, tt, :],
                                     ust[h], zbh)

        # ---- interleaved schedule --------------------------------------
        with tc.tile_pool(name="ps1", bufs=1, space="PSUM") as ps1:
            emit_grp(ps1, 0, act_copies=True)
            emit_scores(0)
            emit_grp(ps1, 1)
            emit_scores(1)
            emit_scores(2)
            emit_grp(ps1, 2)
            emit_scores(3)
            emit_vgen(ps1)

        op = ctx.enter_context(tc.tile_pool(name="op", bufs=4))
        pst = {}

        def emit_proj_mm(pool, co, kts, start, stop):
            if co not in pst:
                pst[co] = pool.tile([128, N], F32, tag=pool is stp and "st"
                                    or "pj", name=f"pjps{co}")
            for kt in kts:
                for n in range(NT):
                    sl = slice(n * 512, (n + 1) * 512)
                    nc.tensor.matmul(
                        pst[co][:, sl],
                        lhsT=projT_sb[:, kt, co * 128:(co + 1) * 128],
                        rhs=Ab[:, kt, sl],
                        start=(start and kt == kts[0]),
                        stop=(stop and kt == kts[-1]),
                        skip_group_check=True)

        def emit_out(co, halves=1):
            ps = pst[co]
            ot = op.tile([128, N], BF16, tag="o", name=f"ot{co}")
            for i in range(halves):
                sl = slice(i * (N // halves), (i + 1) * (N // halves))
                eng = (nc.vector, nc.scalar)[(co + i) % 2]
                if eng is nc.vector:
                    nc.vector.tensor_scalar(
                        out=ot[:, sl], in0=ps[:, sl],
                        scalar1=pbias_sb[:, co:co + 1],
                        scalar2=None, op0=ALU.add)
                else:
                    nc.scalar.activation(ot[:, sl], ps[:, sl], AF.Identity,
                                         bias=pbias_sb[:, co:co + 1],
                                         scale=1.0)
                nc.sync.dma_start(out=t["outT"].ap()[:, co, sl],
                                  in_=ot[:, sl])

        with tc.tile_pool(name="up", bufs=2, space="PSUM") as up:
            backlog = (av_chunks(0, up) + [lambda: emit_zchain(0)]
                       + av_chunks(1, up) + [lambda: emit_zchain(1)])
            emit_scores(4, weave=backlog)
            backlog = (av_chunks(2, up) + [lambda: emit_zchain(2)]
                       + av_chunks(3, up) + [lambda: emit_zchain(3)]
                       + av_chunks(4, up)
                       + [lambda: emit_zchain(4, zbs_eng=nc.scalar)]
                       + av_chunks(5, up)
                       + [lambda: emit_zchain(5, zbs_eng=nc.scalar)])
            emit_scores(5, weave=backlog)
            # tt0/tt1 matmuls of the first two column tiles (stp ring slots)
            # overlap the final Z-chain's DVE work.
            emit_proj_mm(stp, 0, [0, 1], start=True, stop=False)
            emit_proj_mm(stp, 1, [0, 1], start=True, stop=False)
            emit_proj_mm(stp, 0, [2], start=False, stop=True)
            emit_out(0)
            emit_proj_mm(stp, 1, [2], start=False, stop=True)
            emit_out(1)

        # ---- remaining output projection from a dedicated PSUM pool ----
        with tc.tile_pool(name="pj", bufs=2, space="PSUM") as pj:
            for co in range(2, CO):
                emit_proj_mm(pj, co, [0, 1, 2], start=True, stop=True)
                emit_out(co, halves=(2 if co == CO - 1 else 1))


_NC_CACHE = {}


def build_nc(reps: int = 1, with_vbias: bool = False, loop: int = 0,
             q1: int = N, m1s: int = MT):
    key = (reps, with_vbias, loop, q1, m1s)
    if key in _NC_CACHE:
        return _NC_CACHE[key]
    nc = bacc.Bacc("TRN2", target_bir_lowering=False, debug=False,
                   num_devices=N_CORES)
    t = {
        "xT": nc.dram_tensor("xT", [128, KT, N], BF16, kind="ExternalInput"),
        "wqkT": nc.dram_tensor("wqkT", [128, KT, C], BF16,
                               kind="ExternalInput"),
        "qkb": nc.dram_tensor("qkb", [128, KT], F32, kind="ExternalInput"),
        "wvT": nc.dram_tensor("wvT", [128, KT, HPC * HD], BF16,
                              kind="ExternalInput"),
        "vbias": nc.dram_tensor("vbias", [1, HPC * HD], BF16,
                                kind="ExternalInput"),
        "mq": nc.dram_tensor("mq", [2, N], BF16, kind="ExternalInput"),
        "mk": nc.dram_tensor("mk", [2, N], BF16, kind="ExternalInput"),
        "projT": nc.dram_tensor("projT", [128, 3, C], BF16,
                                kind="ExternalInput"),
        "pbias": nc.dram_tensor("pbias", [128, CO], F32,
                                kind="ExternalInput"),
        "outT": nc.dram_tensor("outT", [128, CO, N], BF16,
                               kind="ExternalOutput"),
    }
    with tile.TileContext(nc) as tc:
        if loop:
            with tc.For_i(0, loop, 1):
                _body(nc, tc, t, with_vbias, q1, m1s)
        else:
            for _ in range(reps):
                _body(nc, tc, t, with_vbias, q1, m1s)
    nc.compile()
    _NC_CACHE[key] = nc
    return nc


def _is_onehot(decision: np.ndarray) -> bool:
    vals_ok = np.all((decision == 0.0) | (decision == 1.0))
    return bool(vals_ok and np.all(decision.sum(-1) == 1.0))


def skip_params(decision, S):
    """Conservative shared skip bounds + per-batch token permutations.

    Token order per batch: [template | group2 | group1]. q1 = first query
    column that is group1 in EVERY batch's layout; key tiles >= m1s are
    group1 in every batch. Falls back to dense when the bounds give no
    safely skippable region.
    """
    tpl = N - S
    perms = []
    g2s = []
    for b in range(B):
        g2idx = np.where(decision[b][:, 1] == 1.0)[0]
        g1idx = np.where(decision[b][:, 1] == 0.0)[0]
        perms.append(np.concatenate(
            [np.arange(tpl), tpl + g2idx, tpl + g1idx]))
        g2s.append(len(g2idx))
    q1 = tpl + max(g2s)
    m1s = -(-q1 // 128)          # ceil
    if tpl != 256 or q1 >= N or m1s < 3 or q1 <= 512:
        return N, MT, perms      # dense fallback
    return q1, m1s, perms


def make_in_maps(x, decision, qkv_w, qkv_b, proj_w, proj_b, S, perms):
    in_maps = []
    xT_cache = {}
    for core in range(N_CORES):
        b, hg = core // 2, core % 2
        perm = perms[b]
        if b not in xT_cache:
            xT = np.ascontiguousarray(x[b].T[:, perm])  # [C, N] permuted
            xT_cache[b] = np.ascontiguousarray(
                xT.reshape(KT, 128, N).transpose(1, 0, 2)).astype(BFNP)
        qs = slice(hg * 384, hg * 384 + 384)
        ks = slice(C + hg * 384, C + hg * 384 + 384)
        vs = slice(2 * C + hg * 384, 2 * C + hg * 384 + 384)
        Wqk = np.concatenate([qkv_w[qs] * SCALE, qkv_w[ks]], axis=0)
        bqk = np.concatenate([qkv_b[qs] * SCALE, qkv_b[ks]])
        # j-interleave the 128-row output blocks: [q0|k0|q1|k1|q2|k2] so
        # each head-pair group's weight columns are contiguous in wqkT.
        order = [0, 3, 1, 4, 2, 5]
        Wqk = np.concatenate([Wqk[j * 128:(j + 1) * 128] for j in order])
        bqk = np.concatenate([bqk[j * 128:(j + 1) * 128] for j in order])
        wqkT = np.ascontiguousarray(
            Wqk.T.reshape(KT, 128, C).transpose(1, 0, 2)).astype(BFNP)
        qkb = np.ascontiguousarray(bqk.reshape(KT, 128).T, dtype=np.float32)
        wvT = np.ascontiguousarray(
            qkv_w[vs].T.reshape(KT, 128, 384).transpose(1, 0, 2)).astype(BFNP)
        vbias = qkv_b[vs].reshape(1, 384).astype(BFNP)
        p0 = np.zeros(N, np.float32)
        p0[:N - S] = 1.0
        p1 = np.zeros(N, np.float32)
        p1[N - S:] = decision[b][:, 0]
        p0, p1 = p0[perm], p1[perm]
        mq = np.stack([p1, p0]).astype(BFNP)
        mk = np.stack([-BIG * p0, -BIG * p1]).astype(BFNP)
        projT = np.ascontiguousarray(
            proj_w[:, hg * 384:hg * 384 + 384].T
            .reshape(3, 128, C).transpose(1, 0, 2)).astype(BFNP)
        if hg == 0:
            pbias = np.ascontiguousarray(
                proj_b.reshape(CO, 128).T, dtype=np.float32)
        else:
            pbias = np.zeros((128, CO), np.float32)
        in_maps.append({
            "xT": xT_cache[b], "wqkT": wqkT, "qkb": qkb, "wvT": wvT,
            "vbias": vbias, "mq": mq, "mk": mk,
            "projT": projT, "pbias": pbias,
        })
    return in_maps


def _numpy_fallback(x, decision, qkv_w, qkv_b, proj_w, proj_b, S):
    """Direct port of the reference for non-one-hot policies."""
    EPS = 1e-6
    out = np.empty((B, N, C), np.float32)
    for b in range(B):
        p0 = np.zeros(N, np.float32)
        p0[:N - S] = 1.0
        p1 = np.zeros(N, np.float32)
        p1[N - S:] = decision[b][:, 0]
        p2 = np.zeros(N, np.float32)
        p2[N - S:] = decision[b][:, 1]
        qkv = x[b] @ qkv_w.T + qkv_b
        qkv = qkv.reshape(N, 3, H, HD).transpose(1, 2, 0, 3)
        q, k, v = qkv[0], qkv[1], qkv[2]
        s = p0 + p1 + p2
        ap = (np.outer(s, s) - np.outer(p0, p1) - np.outer(p1, p0))
        ap = ap + (1.0 - ap) * np.eye(N, dtype=np.float32)
        attn = np.einsum('hnd,hmd->hnm', q, k).astype(np.float32) * SCALE
        m = attn.max(-1, keepdims=True)
        e = np.exp(attn - m) * ap[None]
        p = (e + EPS / N) / (e.sum(-1, keepdims=True) + EPS)
        o = np.einsum('hnm,hmd->hnd', p, v)
        out[b] = o.transpose(1, 0, 2).reshape(N, C) @ proj_w.T + proj_b
    return out


def kernel(x, decision, qkv_w, qkv_b, proj_w, proj_b, search_feat_len):
    x = np.asarray(x, np.float32)
    decision = np.asarray(decision, np.float32)
    qkv_w = np.asarray(qkv_w, np.float32)
    qkv_b = np.asarray(qkv_b, np.float32)
    proj_w = np.asarray(proj_w, np.float32)
    proj_b = np.asarray(proj_b, np.float32)
    S = int(np.asarray(search_feat_len))

    if not _is_onehot(decision):
        return _numpy_fallback(x, decision, qkv_w, qkv_b, proj_w, proj_b, S)

    q1, m1s, perms = skip_params(decision, S)
    nc = build_nc(with_vbias=bool(np.any(qkv_b[2 * C:] != 0.0)),
                  q1=q1, m1s=m1s)
    in_maps = make_in_maps(x, decision, qkv_w, qkv_b, proj_w, proj_b, S,
                           perms)
    res = run_bass_kernel_spmd(nc, in_maps, core_ids=list(range(N_CORES)))

    out = np.empty((B, N, C), np.float32)
    for b in range(B):
        partial = (res.results[2 * b]["outT"].astype(np.float32)
                   + res.results[2 * b + 1]["outT"].astype(np.float32))
        out[b][perms[b]] = partial.transpose(1, 0, 2).reshape(C, N).T
    return out


# revision 5
# speedup vs baseline: 1.3861x; 1.3861x over previous
"""Trainium2 Bass kernel for policy-masked attention (sparse_attention), v3.

Shapes (hardcoded): x [4,1024,768], decision [4,768,2], qkv_w [2304,768],
qkv_b [2304], proj_w [768,768], proj_b [768], search_feat_len=768.

Sharding: 8 cores = 4 batches x 2 head-groups (6 heads each). Each core
computes its batch's q/k/v for its heads, the policy-masked softmax
(one-hot policy folded into the score matmul as 2 extra contraction rows
of -BIG * indicator outer products), attn @ v with a fused ones-column
producing the softmax denominator, and a partial output projection.
Host sums the two head-group partials per batch.

v3 design:
- all PE operands bf16 (same PE rate as fp32r, half DMA/SBUF).
- host sorts tokens [template | group2 | group1] per batch, so the two
  masked blocks (template-q x group1-k and group1-q x template-k) become
  contiguous; score/exp/AV skip the conservatively-safe common region
  (bounds shared across cores: q1 = tpl + max_b g2). The rank-2 -BIG mask
  rows still handle all boundary tiles exactly.
- interleaved emission: QK-gen j-groups and the first four heads' score
  streams alternate, so the scalar engine's exp stream starts ~6us in and
  runs gapless; AV matmuls for those heads run later against buffered e
  tiles. V-gen sits between; proj tt0/tt1 matmuls overlap the last
  Z-chain.
- softmax 1/Z: DVE reciprocal of the fused Z row, PE ones-matmul
  broadcast, one PSUM->SBUF stage (single-PSUM-operand rule), DVE muls.
- AV accumulation emits the full-width m=2 tile first so every PSUM
  element is has_written-initialized regardless of clear granularity.
"""
import numpy as np
import ml_dtypes

import concourse.bass as bass
import concourse.tile as tile
from concourse import bacc, mybir
from concourse.bass_utils import run_bass_kernel_spmd

F32 = mybir.dt.float32
BF16 = mybir.dt.bfloat16
AF = mybir.ActivationFunctionType
ALU = mybir.AluOpType
BFNP = ml_dtypes.bfloat16

B, N, C = 4, 1024, 768
H = 12
HD = 64
HPC = 6              # heads per core
KT = C // 128        # 6 contraction tiles
NT = N // 512        # 2 moving slices
MT = N // 128        # 8 key tiles
CO = C // 128        # 6 output-column tiles
SCALE = HD ** -0.5
BIG = 32768.0
N_CORES = 8


def _score_ranges(m, q1, m1s):
    """Query-column ranges to compute for key-tile m (512-col chunks)."""
    if m < 2 and q1 < N:          # template keys: skip group1 queries
        return [(0, 512), (512, q1)] if q1 > 512 else [(0, q1)]
    if m >= m1s:                  # group1 keys: skip template queries
        return [(256, 512), (512, 1024)]
    return [(0, 512), (512, 1024)]


def _exp_range(m, q1, m1s):
    if m < 2 and q1 < N:
        return (0, q1)
    if m >= m1s:
        return (256, N)
    return (0, N)


def _body(nc, tc, t, with_vbias, q1, m1s, warmup=True):
    import contextlib
    with contextlib.ExitStack() as ctx:
        consts = ctx.enter_context(tc.tile_pool(name="consts", bufs=1))
        headp = ctx.enter_context(tc.tile_pool(name="headp", bufs=1))

        xT_sb = consts.tile([128, KT, N], BF16)
        wqkT_sb = consts.tile([128, KT, C], BF16)
        wvT_sb = consts.tile([128, KT, HPC * HD], BF16)
        vbias_sb = consts.tile([1, HPC * HD], BF16)
        ones1_sb = consts.tile([1, 128], BF16)
        ones64_sb = consts.tile([1, 64], BF16)
        qkb_sb = consts.tile([128, KT], F32)
        projT_sb = consts.tile([128, 3, C], BF16)
        pbias_sb = consts.tile([128, CO], F32)

        # Per-head q/k tiles [66, N]: rows 0-63 head data, rows 64-65 the
        # rank-2 log-mask factors (k side: -BIG*p0,-BIG*p1; q side: p1,p0).
        qh = [headp.tile([66, N], BF16, name=f"qh{h}", tag=f"qh{h}")
              for h in range(HPC)]
        kh = [headp.tile([66, N], BF16, name=f"kh{h}", tag=f"kh{h}")
              for h in range(HPC)]

        # Every dma_start costs ~1.2us of SEQ time on the issuing engine, so
        # batch the inputs into few large DMAs and keep the scalar (ACT)
        # queue nearly empty -- its sequencer must be free for the exp
        # stream. wqkT is stored j-interleaved ([j0|j3|j1|j4|j2|j5]) so each
        # group's 256 weight columns are one strided DMA; x lands in three
        # ascending pieces so group 0's kt-outer matmuls start early.
        nc.scalar.dma_start(out=wqkT_sb[:, :, 0:256],
                            in_=t["wqkT"].ap()[:, :, 0:256])
        nc.scalar.dma_start(out=qh[0][64:66, :], in_=t["mq"].ap())
        nc.scalar.dma_start(out=qkb_sb, in_=t["qkb"].ap())
        if with_vbias:
            nc.scalar.dma_start(out=vbias_sb, in_=t["vbias"].ap())
        nc.sync.dma_start(out=xT_sb[:, 0, :], in_=t["xT"].ap()[:, 0, :])
        nc.sync.dma_start(out=kh[0][64:66, :], in_=t["mk"].ap())
        nc.sync.dma_start(out=xT_sb[:, 1:3, :], in_=t["xT"].ap()[:, 1:3, :])
        nc.sync.dma_start(out=xT_sb[:, 3:6, :], in_=t["xT"].ap()[:, 3:6, :])
        nc.sync.dma_start(out=wqkT_sb[:, :, 256:768],
                          in_=t["wqkT"].ap()[:, :, 256:768])
        nc.sync.dma_start(out=wvT_sb, in_=t["wvT"].ap())
        for h in range(1, HPC):
            nc.sync.dma_start(out=qh[h][64:66, :], in_=t["mq"].ap())
            nc.sync.dma_start(out=kh[h][64:66, :], in_=t["mk"].ap())
        nc.sync.dma_start(out=pbias_sb, in_=t["pbias"].ap())
        nc.sync.dma_start(out=projT_sb, in_=t["projT"].ap())
        nc.vector.memset(ones1_sb, 1.0)
        nc.vector.memset(ones64_sb, 1.0)

        # V in token-major layout with a fused ones column: [128, MT, 6*65]
        V_sb = consts.tile([128, MT, HPC * 65], BF16)
        vv = V_sb.rearrange("p m (h e) -> p m h e", h=HPC)
        nc.vector.memset(vv[:, :, :, 64:65], 1.0)

        # Pre-load the Exp activation table off the critical path.
        warm = consts.tile([1, 1], F32)
        nc.scalar.activation(warm, qkb_sb[0:1, 0:1], AF.Exp)


        abp = ctx.enter_context(tc.tile_pool(name="abp", bufs=1))
        Ab = abp.tile([128, 3, N], BF16)

        stp = ctx.enter_context(tc.tile_pool(name="stp", bufs=2, space="PSUM"))

        # Warm up the PE p-state/HAM during the initial DMA window: ~4.5us
        # of dummy matmuls on memset constants so the real QK-gen matmuls
        # run at full clock. Serialized WAW on one PSUM tile keeps them
        # back-to-back; one DVE read is the ring consumer.
        if warmup:
            wrhs = consts.tile([1, 512], BF16)
            nc.vector.memset(wrhs, 0.5)
            wps = stp.tile([64, 512], F32, tag="st", name="wps")
            for _ in range(9):
                nc.tensor.matmul(wps, lhsT=ones64_sb, rhs=wrhs,
                                 start=True, stop=True)
            wsink = consts.tile([1, 1], F32)
            nc.vector.tensor_copy(wsink, wps[0:1, 0:1])

        ep = ctx.enter_context(tc.tile_pool(name="ep", bufs=48))
        zp = ctx.enter_context(tc.tile_pool(name="zp", bufs=4))
        zdram = ctx.enter_context(tc.tile_pool(name="zdram", bufs=1,
                                               space="DRAM"))
        zd = zdram.tile([4, N], BF16)

        ups, zrec, ust, etiles = {}, {}, {}, {}

        # ---- emission helpers ------------------------------------------
        def emit_grp(ps1, g, act_copies=False):
            """QK-gen for j-group {g, g+3} -> heads 2g, 2g+1. kt-outer and
            n-inner: all four [128,512] PSUM tiles accumulate together, so
            the matmuls pace with the x/w DMA chunk stream. wqkT column
            block 2g holds the q rows, 2g+1 the k rows (j-interleaved)."""
            for i, (p, n) in enumerate(
                    (p, n) for p in (2 * g, 2 * g + 1) for n in range(NT)):
                ps = ps1.tile([128, 512], F32, bufs=1,
                              tag=f"g{'qk'[p % 2]}{n}", name=f"g{g}_{p}_{n}")
                for kt in range(KT):
                    nc.tensor.matmul(
                        ps,
                        lhsT=wqkT_sb[:, kt, p * 128:(p + 1) * 128],
                        rhs=xT_sb[:, kt, n * 512:(n + 1) * 512],
                        start=(kt == 0), stop=(kt == KT - 1))
                tiles = qh if p % 2 == 0 else kh
                sl = slice(n * 512, (n + 1) * 512)
                for half in range(2):
                    h = 2 * g + half
                    dst = tiles[h][0:64, sl]
                    srcap = ps[half * 64:(half + 1) * 64, :]
                    bias = qkb_sb[half * 64:(half + 1) * 64, p:p + 1]
                    if act_copies and (i + half) % 2 == 1:
                        nc.scalar.activation(dst, srcap, AF.Identity,
                                             bias=bias, scale=1.0)
                    else:
                        nc.vector.tensor_scalar(out=dst, in0=srcap,
                                                scalar1=bias, scalar2=None,
                                                op0=ALU.add)

        def emit_vgen(ps1):
            for m in range(MT):
                psv = ps1.tile([128, HPC * HD], F32,
                               tag=f"gq{m % 2}", bufs=1)
                for kt in range(KT):
                    nc.tensor.matmul(psv,
                                     lhsT=xT_sb[:, kt, m * 128:(m + 1) * 128],
                                     rhs=wvT_sb[:, kt, :],
                                     start=(kt == 0),
                                     stop=(not with_vbias and kt == KT - 1))
                if with_vbias:
                    nc.tensor.matmul(psv, lhsT=ones1_sb, rhs=vbias_sb,
                                     start=False, stop=True)
                nc.vector.tensor_copy(vv[:, m, :, 0:64],
                                      psv.rearrange("p (h d) -> p h d", h=HPC))

        def emit_scores(h, weave=()):
            """All 8 key-tiles of head h: score matmuls + exp. Up to two
            backlog emitters (AV matmuls / Z-chains) are woven in after each
            key-tile so the PE fills the exp-paced slack."""
            wq = list(weave)
            for m in range(MT):
                st = stp.tile([128, N], F32, tag="st", name=f"st{h}_{m}")
                for a, b in _score_ranges(m, q1, m1s):
                    nc.tensor.matmul(st[:, a:b],
                                     lhsT=kh[h][:, m * 128:(m + 1) * 128],
                                     rhs=qh[h][:, a:b],
                                     start=True, stop=True,
                                     skip_group_check=True)
                e = ep.tile([128, N], BF16, tag="e", name=f"e{h}_{m}")
                ea, eb = _exp_range(m, q1, m1s)
                nc.scalar.activation(e[:, ea:eb], st[:, ea:eb], AF.Exp)
                etiles[(h, m)] = e
                for _ in range(2):
                    if wq:
                        wq.pop(0)()
            for f in wq:
                f()

        def av_chunks(h, up):
            """Per-key-tile emitters for head h's AV accumulation. m=2
            (always full-width) goes first so every PSUM element is
            initialized by a start-group matmul; the partial-width tiles
            then accumulate per-element. The final emitter computes 1/Z
            (EPS is negligible: Z >= exp(s_ii); bf16 is a 0.4% common-mode
            scale on one head-query)."""
            order = [2] + [m for m in range(MT) if m != 2]

            def mk(i, m):
                def f():
                    if i == 0:
                        ups[h] = up.tile([65, N], F32, name=f"u{h}", tag="u")
                    e = etiles.pop((h, m))
                    rs = _score_ranges(m, q1, m1s)
                    for k, (a, b) in enumerate(rs):
                        nc.tensor.matmul(
                            ups[h][:, a:b],
                            lhsT=V_sb[:, m, h * 65:(h + 1) * 65],
                            rhs=e[:, a:b],
                            start=(i == 0),
                            stop=(i == len(order) - 1 and k == len(rs) - 1),
                            skip_group_check=True)
                return f

            def zf():
                # recip (DVE) and the eager U->SBUF copy (ACT for the last
                # pair, DVE otherwise) are the only ups readers, so the
                # ups ring slot recycles ~2.4us sooner than a PSUM-side
                # normalize would allow.
                zrec[h] = zp.tile([1, N], BF16, tag="z", name=f"z{h}")
                with nc.allow_low_precision(reason="1/Z common-mode scale"):
                    nc.vector.reciprocal(zrec[h], ups[h][64:65, :])
                ust[h] = zp.tile([64, N], BF16, tag="ust", name=f"ust{h}")
                if h >= 4:
                    nc.scalar.activation(ust[h], ups[h][0:64, :], AF.Copy)
                else:
                    nc.vector.tensor_copy(ust[h], ups[h][0:64, :])

            return [mk(i, m) for i, m in enumerate(order)] + [zf]

        def emit_av(h, up):
            for f in av_chunks(h, up):
                f()

        def emit_zchain(h, zbs_eng=None):
            """Per-head normalize: Ab = Ust * broadcast(1/Z). Heads 0-3
            broadcast 1/Z across partitions via a DRAM bounce (no PSUM slot,
            no PE work; latency hides behind phase 2). The last pair is
            latency-critical and uses a PE ones-matmul into an stp slot."""
            tt, base = h // 2, 64 * (h % 2)
            if h < 4:
                nc.sync.dma_start(out=zd[h:h + 1, :], in_=zrec[h])
                zsrc = zd[h:h + 1, :]
                bsrc = bass.AP(tensor=zsrc.tensor, offset=zsrc.offset,
                               ap=[[0, 64]] + list(zsrc.ap[1:]))
                zbs = zp.tile([64, N], BF16, tag="zbs", name=f"zbs{h}")
                nc.sync.dma_start(out=zbs, in_=bsrc)
                nc.vector.tensor_mul(Ab[base:base + 64, tt, :],
                                     ust[h], zbs)
                return
            zbh = stp.tile([64, N], F32, tag="st", name=f"zbh{h}")
            for n in range(NT):
                sl = slice(n * 512, (n + 1) * 512)
                nc.tensor.matmul(zbh[:, sl], lhsT=ones64_sb,
                                 rhs=zrec[h][:, sl], start=True, stop=True)
            if h == HPC - 1:
                # split so the first proj tt2 matmuls start half a mul early
                for n in range(NT):
                    sl = slice(n * 512, (n + 1) * 512)
                    nc.vector.tensor_mul(Ab[base:base + 64, tt, sl],
                                         ust[h][:, sl], zbh[:, sl])
            else:
                nc.vector.tensor_mul(Ab[base:base + 64, tt, :],
                                     ust[h], zbh)

        # ---- interleaved schedule --------------------------------------
        with tc.tile_pool(name="ps1", bufs=1, space="PSUM") as ps1:
            emit_grp(ps1, 0, act_copies=True)
            emit_scores(0)
            emit_grp(ps1, 1)
            emit_scores(1)
            emit_scores(2)
            emit_grp(ps1, 2)
            emit_scores(3)
            emit_vgen(ps1)

        op = ctx.enter_context(tc.tile_pool(name="op", bufs=4))
        pst = {}

        def emit_proj_mm(pool, co, kts, start, stop):
            if co not in pst:
                pst[co] = pool.tile([128, N], F32, tag=pool is stp and "st"
                                    or "pj", name=f"pjps{co}")
            for kt in kts:
                for n in range(NT):
                    sl = slice(n * 512, (n + 1) * 512)
                    nc.tensor.matmul(
                        pst[co][:, sl],
                        lhsT=projT_sb[:, kt, co * 128:(co + 1) * 128],
                        rhs=Ab[:, kt, sl],
                        start=(start and kt == kts[0]),
                        stop=(stop and kt == kts[-1]),
                        skip_group_check=True)

        def emit_out(co, halves=1):
            ps = pst[co]
            ot = op.tile([128, N], BF16, tag="o", name=f"ot{co}")
            for i in range(halves):
                sl = slice(i * (N // halves), (i + 1) * (N // halves))
                eng = (nc.vector, nc.scalar)[(co + i) % 2]
                if eng is nc.vector:
                    nc.vector.tensor_scalar(
                        out=ot[:, sl], in0=ps[:, sl],
                        scalar1=pbias_sb[:, co:co + 1],
                        scalar2=None, op0=ALU.add)
                else:
                    nc.scalar.activation(ot[:, sl], ps[:, sl], AF.Identity,
                                         bias=pbias_sb[:, co:co + 1],
                                         scale=1.0)
                nc.sync.dma_start(out=t["outT"].ap()[:, co, sl],
                                  in_=ot[:, sl])

        with tc.tile_pool(name="up", bufs=2, space="PSUM") as up:
            backlog = (av_chunks(0, up) + [lambda: emit_zchain(0)]
                       + av_chunks(1, up) + [lambda: emit_zchain(1)])
            emit_scores(4, weave=backlog)
            backlog = (av_chunks(2, up) + [lambda: emit_zchain(2)]
                       + av_chunks(3, up) + [lambda: emit_zchain(3)]
                       + av_chunks(4, up)
                       + [lambda: emit_zchain(4, zbs_eng=nc.scalar)]
                       + av_chunks(5, up))
            emit_scores(5, weave=backlog)
            # co0's tt0/tt1 matmuls (stp ring slot, allocated before zbh5 to
            # keep the ring deadlock-free) fill the final reciprocal's
            # latency; the last pair's Z-chain follows.
            emit_proj_mm(stp, 0, [0, 1], start=True, stop=False)
            emit_zchain(5, zbs_eng=nc.scalar)
        # `up` closes once recip5/Ust5 are done, freeing banks for pj while
        # the mul5 chain drains.
        with tc.tile_pool(name="pj", bufs=2, space="PSUM") as pj:
            for co in (1, 2):
                emit_proj_mm(pj, co, [0, 1], start=True, stop=False)
            emit_proj_mm(stp, 0, [2], start=False, stop=True)
            emit_out(0)
            for co in (1, 2):
                emit_proj_mm(pj, co, [2], start=False, stop=True)
                emit_out(co)
            for co in range(3, CO):
                emit_proj_mm(pj, co, [0, 1, 2], start=True, stop=True)
                emit_out(co, halves=(2 if co == CO - 1 else 1))


_NC_CACHE = {}


def build_nc(reps: int = 1, with_vbias: bool = False, loop: int = 0,
             q1: int = N, m1s: int = MT):
    key = (reps, with_vbias, loop, q1, m1s)
    if key in _NC_CACHE:
        return _NC_CACHE[key]
    nc = bacc.Bacc("TRN2", target_bir_lowering=False, debug=False,
                   num_devices=N_CORES)
    t = {
        "xT": nc.dram_tensor("xT", [128, KT, N], BF16, kind="ExternalInput"),
        "wqkT": nc.dram_tensor("wqkT", [128, KT, C], BF16,
                               kind="ExternalInput"),
        "qkb": nc.dram_tensor("qkb", [128, KT], F32, kind="ExternalInput"),
        "wvT": nc.dram_tensor("wvT", [128, KT, HPC * HD], BF16,
                              kind="ExternalInput"),
        "vbias": nc.dram_tensor("vbias", [1, HPC * HD], BF16,
                                kind="ExternalInput"),
        "mq": nc.dram_tensor("mq", [2, N], BF16, kind="ExternalInput"),
        "mk": nc.dram_tensor("mk", [2, N], BF16, kind="ExternalInput"),
        "projT": nc.dram_tensor("projT", [128, 3, C], BF16,
                                kind="ExternalInput"),
        "pbias": nc.dram_tensor("pbias", [128, CO], F32,
                                kind="ExternalInput"),
        "outT": nc.dram_tensor("outT", [128, CO, N], BF16,
                               kind="ExternalOutput"),
    }
    with tile.TileContext(nc) as tc:
        if loop:
            with tc.For_i(0, loop, 1):
                _body(nc, tc, t, with_vbias, q1, m1s, warmup=False)
        else:
            for _ in range(reps):
                _body(nc, tc, t, with_vbias, q1, m1s)
    nc.compile()
    _NC_CACHE[key] = nc
    return nc


def _is_onehot(decision: np.ndarray) -> bool:
    vals_ok = np.all((decision == 0.0) | (decision == 1.0))
    return bool(vals_ok and np.all(decision.sum(-1) == 1.0))


def skip_params(decision, S):
    """Conservative shared skip bounds + per-batch token permutations.

    Token order per batch: [template | group2 | group1]. q1 = first query
    column that is group1 in EVERY batch's layout; key tiles >= m1s are
    group1 in every batch. Falls back to dense when the bounds give no
    safely skippable region.
    """
    tpl = N - S
    perms = []
    g2s = []
    for b in range(B):
        g2idx = np.where(decision[b][:, 1] == 1.0)[0]
        g1idx = np.where(decision[b][:, 1] == 0.0)[0]
        perms.append(np.concatenate(
            [np.arange(tpl), tpl + g2idx, tpl + g1idx]))
        g2s.append(len(g2idx))
    q1 = tpl + max(g2s)
    m1s = -(-q1 // 128)          # ceil
    if tpl != 256 or q1 >= N or m1s < 3 or q1 <= 512:
        return N, MT, perms      # dense fallback
    return q1, m1s, perms


def make_in_maps(x, decision, qkv_w, qkv_b, proj_w, proj_b, S, perms):
    in_maps = []
    xT_cache = {}
    for core in range(N_CORES):
        b, hg = core // 2, core % 2
        perm = perms[b]
        if b not in xT_cache:
            xT = np.ascontiguousarray(x[b].T[:, perm])  # [C, N] permuted
            xT_cache[b] = np.ascontiguousarray(
                xT.reshape(KT, 128, N).transpose(1, 0, 2)).astype(BFNP)
        qs = slice(hg * 384, hg * 384 + 384)
        ks = slice(C + hg * 384, C + hg * 384 + 384)
        vs = slice(2 * C + hg * 384, 2 * C + hg * 384 + 384)
        Wqk = np.concatenate([qkv_w[qs] * SCALE, qkv_w[ks]], axis=0)
        bqk = np.concatenate([qkv_b[qs] * SCALE, qkv_b[ks]])
        # j-interleave the 128-row output blocks: [q0|k0|q1|k1|q2|k2] so
        # each head-pair group's weight columns are contiguous in wqkT.
        order = [0, 3, 1, 4, 2, 5]
        Wqk = np.concatenate([Wqk[j * 128:(j + 1) * 128] for j in order])
        bqk = np.concatenate([bqk[j * 128:(j + 1) * 128] for j in order])
        wqkT = np.ascontiguousarray(
            Wqk.T.reshape(KT, 128, C).transpose(1, 0, 2)).astype(BFNP)
        qkb = np.ascontiguousarray(bqk.reshape(KT, 128).T, dtype=np.float32)
        wvT = np.ascontiguousarray(
            qkv_w[vs].T.reshape(KT, 128, 384).transpose(1, 0, 2)).astype(BFNP)
        vbias = qkv_b[vs].reshape(1, 384).astype(BFNP)
        p0 = np.zeros(N, np.float32)
        p0[:N - S] = 1.0
        p1 = np.zeros(N, np.float32)
        p1[N - S:] = decision[b][:, 0]
        p0, p1 = p0[perm], p1[perm]
        mq = np.stack([p1, p0]).astype(BFNP)
        mk = np.stack([-BIG * p0, -BIG * p1]).astype(BFNP)
        projT = np.ascontiguousarray(
            proj_w[:, hg * 384:hg * 384 + 384].T
            .reshape(3, 128, C).transpose(1, 0, 2)).astype(BFNP)
        if hg == 0:
            pbias = np.ascontiguousarray(
                proj_b.reshape(CO, 128).T, dtype=np.float32)
        else:
            pbias = np.zeros((128, CO), np.float32)
        in_maps.append({
            "xT": xT_cache[b], "wqkT": wqkT, "qkb": qkb, "wvT": wvT,
            "vbias": vbias, "mq": mq, "mk": mk,
            "projT": projT, "pbias": pbias,
        })
    return in_maps


def _numpy_fallback(x, decision, qkv_w, qkv_b, proj_w, proj_b, S):
    """Direct port of the reference for non-one-hot policies."""
    EPS = 1e-6
    out = np.empty((B, N, C), np.float32)
    for b in range(B):
        p0 = np.zeros(N, np.float32)
        p0[:N - S] = 1.0
        p1 = np.zeros(N, np.float32)
        p1[N - S:] = decision[b][:, 0]
        p2 = np.zeros(N, np.float32)
        p2[N - S:] = decision[b][:, 1]
        qkv = x[b] @ qkv_w.T + qkv_b
        qkv = qkv.reshape(N, 3, H, HD).transpose(1, 2, 0, 3)
        q, k, v = qkv[0], qkv[1], qkv[2]
        s = p0 + p1 + p2
        ap = (np.outer(s, s) - np.outer(p0, p1) - np.outer(p1, p0))
        ap = ap + (1.0 - ap) * np.eye(N, dtype=np.float32)
        attn = np.einsum('hnd,hmd->hnm', q, k).astype(np.float32) * SCALE
        m = attn.max(-1, keepdims=True)
        e = np.exp(attn - m) * ap[None]
        p = (e + EPS / N) / (e.sum(-1, keepdims=True) + EPS)
        o = np.einsum('hnm,hmd->hnd', p, v)
        out[b] = o.transpose(1, 0, 2).reshape(N, C) @ proj_w.T + proj_b
    return out


def kernel(x, decision, qkv_w, qkv_b, proj_w, proj_b, search_feat_len):
    x = np.asarray(x, np.float32)
    decision = np.asarray(decision, np.float32)
    qkv_w = np.asarray(qkv_w, np.float32)
    qkv_b = np.asarray(qkv_b, np.float32)
    proj_w = np.asarray(proj_w, np.float32)
    proj_b = np.asarray(proj_b, np.float32)
    S = int(np.asarray(search_feat_len))

    if not _is_onehot(decision):
        return _numpy_fallback(x, decision, qkv_w, qkv_b, proj_w, proj_b, S)

    q1, m1s, perms = skip_params(decision, S)
    nc = build_nc(with_vbias=bool(np.any(qkv_b[2 * C:] != 0.0)),
                  q1=q1, m1s=m1s)
    in_maps = make_in_maps(x, decision, qkv_w, qkv_b, proj_w, proj_b, S,
                           perms)
    res = run_bass_kernel_spmd(nc, in_maps, core_ids=list(range(N_CORES)))

    out = np.empty((B, N, C), np.float32)
    for b in range(B):
        partial = (res.results[2 * b]["outT"].astype(np.float32)
                   + res.results[2 * b + 1]["outT"].astype(np.float32))
        out[b][perms[b]] = partial.transpose(1, 0, 2).reshape(C, N).T
    return out


# revision 10
# speedup vs baseline: 1.3898x; 1.0027x over previous
"""Trainium2 Bass kernel for policy-masked attention (sparse_attention), v3.

Shapes (hardcoded): x [4,1024,768], decision [4,768,2], qkv_w [2304,768],
qkv_b [2304], proj_w [768,768], proj_b [768], search_feat_len=768.

Sharding: 8 cores = 4 batches x 2 head-groups (6 heads each). Each core
computes its batch's q/k/v for its heads, the policy-masked softmax
(one-hot policy folded into the score matmul as 2 extra contraction rows
of -BIG * indicator outer products), attn @ v with a fused ones-column
producing the softmax denominator, and a partial output projection.
Host sums the two head-group partials per batch.

v3 design:
- all PE operands bf16 (same PE rate as fp32r, half DMA/SBUF).
- host sorts tokens [template | group2 | group1] per batch, so the two
  masked blocks (template-q x group1-k and group1-q x template-k) become
  contiguous; score/exp/AV skip the conservatively-safe common region
  (bounds shared across cores: q1 = tpl + max_b g2). The rank-2 -BIG mask
  rows still handle all boundary tiles exactly.
- interleaved emission: QK-gen j-groups and the first four heads' score
  streams alternate, so the scalar engine's exp stream starts ~6us in and
  runs gapless; AV matmuls for those heads run later against buffered e
  tiles. V-gen sits between; proj tt0/tt1 matmuls overlap the last
  Z-chain.
- softmax 1/Z: DVE reciprocal of the fused Z row, PE ones-matmul
  broadcast, one PSUM->SBUF stage (single-PSUM-operand rule), DVE muls.
- AV accumulation emits the full-width m=2 tile first so every PSUM
  element is has_written-initialized regardless of clear granularity.
"""
import numpy as np
import ml_dtypes

import concourse.bass as bass
import concourse.tile as tile
from concourse import bacc, mybir
from concourse.bass_utils import run_bass_kernel_spmd

F32 = mybir.dt.float32
BF16 = mybir.dt.bfloat16
AF = mybir.ActivationFunctionType
ALU = mybir.AluOpType
BFNP = ml_dtypes.bfloat16

B, N, C = 4, 1024, 768
H = 12
HD = 64
HPC = 6              # heads per core
KT = C // 128        # 6 contraction tiles
NT = N // 512        # 2 moving slices
MT = N // 128        # 8 key tiles
CO = C // 128        # 6 output-column tiles
SCALE = HD ** -0.5
BIG = 32768.0
N_CORES = 8


def _score_ranges(m, q1, m1s):
    """Query-column ranges to compute for key-tile m (512-col chunks)."""
    if m < 2 and q1 < N:          # template keys: skip group1 queries
        return [(0, 512), (512, q1)] if q1 > 512 else [(0, q1)]
    if m >= m1s:                  # group1 keys: skip template queries
        return [(256, 512), (512, 1024)]
    return [(0, 512), (512, 1024)]


def _exp_range(m, q1, m1s):
    if m < 2 and q1 < N:
        return (0, q1)
    if m >= m1s:
        return (256, N)
    return (0, N)


def _body(nc, tc, t, with_vbias, q1, m1s, warmup=True):
    import contextlib
    with contextlib.ExitStack() as ctx:
        consts = ctx.enter_context(tc.tile_pool(name="consts", bufs=1))
        headp = ctx.enter_context(tc.tile_pool(name="headp", bufs=1))

        xT_sb = consts.tile([128, KT, N], BF16)
        wqkT_sb = consts.tile([128, KT, C], BF16)
        wvT_sb = consts.tile([128, KT, HPC * HD], BF16)
        vbias_sb = consts.tile([1, HPC * HD], BF16)
        ones1_sb = consts.tile([1, 128], BF16)
        ones64_sb = consts.tile([1, 64], BF16)
        qkb_sb = consts.tile([128, KT], F32)
        projT_sb = consts.tile([128, 3, C], BF16)
        pbias_sb = consts.tile([128, CO], F32)

        # Per-head q/k tiles [66, N]: rows 0-63 head data, rows 64-65 the
        # rank-2 log-mask factors (k side: -BIG*p0,-BIG*p1; q side: p1,p0).
        qh = [headp.tile([66, N], BF16, name=f"qh{h}", tag=f"qh{h}")
              for h in range(HPC)]
        kh = [headp.tile([66, N], BF16, name=f"kh{h}", tag=f"kh{h}")
              for h in range(HPC)]

        # Every dma_start costs ~1.2us of SEQ time on the issuing engine, so
        # batch the inputs into few large DMAs and keep the scalar (ACT)
        # queue nearly empty -- its sequencer must be free for the exp
        # stream. wqkT is stored j-interleaved ([j0|j3|j1|j4|j2|j5]) so each
        # group's 256 weight columns are one strided DMA; x lands in three
        # ascending pieces so group 0's kt-outer matmuls start early.
        nc.scalar.dma_start(out=wqkT_sb[:, :, 0:256],
                            in_=t["wqkT"].ap()[:, :, 0:256])
        nc.scalar.dma_start(out=qh[0][64:66, :], in_=t["mq"].ap())
        nc.scalar.dma_start(out=qkb_sb, in_=t["qkb"].ap())
        if with_vbias:
            nc.scalar.dma_start(out=vbias_sb, in_=t["vbias"].ap())
        nc.sync.dma_start(out=xT_sb[:, 0, :], in_=t["xT"].ap()[:, 0, :])
        nc.sync.dma_start(out=kh[0][64:66, :], in_=t["mk"].ap())
        nc.sync.dma_start(out=xT_sb[:, 1:3, :], in_=t["xT"].ap()[:, 1:3, :])
        nc.sync.dma_start(out=xT_sb[:, 3:6, :], in_=t["xT"].ap()[:, 3:6, :])
        nc.sync.dma_start(out=wqkT_sb[:, :, 256:768],
                          in_=t["wqkT"].ap()[:, :, 256:768])
        nc.sync.dma_start(out=wvT_sb, in_=t["wvT"].ap())
        for h in range(1, HPC):
            nc.sync.dma_start(out=qh[h][64:66, :], in_=t["mq"].ap())
            nc.sync.dma_start(out=kh[h][64:66, :], in_=t["mk"].ap())
        nc.sync.dma_start(out=pbias_sb, in_=t["pbias"].ap())
        nc.sync.dma_start(out=projT_sb, in_=t["projT"].ap())
        nc.vector.memset(ones1_sb, 1.0)
        nc.vector.memset(ones64_sb, 1.0)

        # V in token-major layout with a fused ones column: [128, MT, 6*65]
        V_sb = consts.tile([128, MT, HPC * 65], BF16)
        vv = V_sb.rearrange("p m (h e) -> p m h e", h=HPC)
        nc.vector.memset(vv[:, :, :, 64:65], 1.0)

        # Pre-load the Exp activation table off the critical path.
        warm = consts.tile([1, 1], F32)
        nc.scalar.activation(warm, qkb_sb[0:1, 0:1], AF.Exp)


        abp = ctx.enter_context(tc.tile_pool(name="abp", bufs=1))
        Ab = abp.tile([128, 3, N], BF16)

        stp = ctx.enter_context(tc.tile_pool(name="stp", bufs=2, space="PSUM"))

        # Warm up the PE p-state/HAM during the initial DMA window: ~4.5us
        # of dummy matmuls on memset constants so the real QK-gen matmuls
        # run at full clock. Serialized WAW on one PSUM tile keeps them
        # back-to-back; one DVE read is the ring consumer.
        if warmup:
            wrhs = consts.tile([1, 512], BF16)
            nc.vector.memset(wrhs, 0.5)
            wps = stp.tile([64, 512], F32, tag="st", name="wps")
            for _ in range(9):
                nc.tensor.matmul(wps, lhsT=ones64_sb, rhs=wrhs,
                                 start=True, stop=True)
            wsink = consts.tile([1, 1], F32)
            nc.vector.tensor_copy(wsink, wps[0:1, 0:1])

        ep = ctx.enter_context(tc.tile_pool(name="ep", bufs=48))
        zp = ctx.enter_context(tc.tile_pool(name="zp", bufs=4))
        zdram = ctx.enter_context(tc.tile_pool(name="zdram", bufs=1,
                                               space="DRAM"))
        zd = zdram.tile([4, N], BF16)

        ups, zrec, ust, etiles = {}, {}, {}, {}

        # ---- emission helpers ------------------------------------------
        def emit_grp(ps1, g, act_copies=False):
            """QK-gen for j-group {g, g+3} -> heads 2g, 2g+1. kt-outer and
            n-inner: all four [128,512] PSUM tiles accumulate together, so
            the matmuls pace with the x/w DMA chunk stream. wqkT column
            block 2g holds the q rows, 2g+1 the k rows (j-interleaved)."""
            for i, (p, n) in enumerate(
                    (p, n) for p in (2 * g, 2 * g + 1) for n in range(NT)):
                ps = ps1.tile([128, 512], F32, bufs=1,
                              tag=f"g{'qk'[p % 2]}{n}", name=f"g{g}_{p}_{n}")
                for kt in range(KT):
                    nc.tensor.matmul(
                        ps,
                        lhsT=wqkT_sb[:, kt, p * 128:(p + 1) * 128],
                        rhs=xT_sb[:, kt, n * 512:(n + 1) * 512],
                        start=(kt == 0), stop=(kt == KT - 1))
                tiles = qh if p % 2 == 0 else kh
                sl = slice(n * 512, (n + 1) * 512)
                for half in range(2):
                    h = 2 * g + half
                    dst = tiles[h][0:64, sl]
                    srcap = ps[half * 64:(half + 1) * 64, :]
                    bias = qkb_sb[half * 64:(half + 1) * 64, p:p + 1]
                    if act_copies and (i + half) % 2 == 1:
                        nc.scalar.activation(dst, srcap, AF.Identity,
                                             bias=bias, scale=1.0)
                    else:
                        nc.vector.tensor_scalar(out=dst, in0=srcap,
                                                scalar1=bias, scalar2=None,
                                                op0=ALU.add)

        def emit_vgen(ps1):
            for m in range(MT):
                psv = ps1.tile([128, HPC * HD], F32,
                               tag=f"gq{m % 2}", bufs=1)
                for kt in range(KT):
                    nc.tensor.matmul(psv,
                                     lhsT=xT_sb[:, kt, m * 128:(m + 1) * 128],
                                     rhs=wvT_sb[:, kt, :],
                                     start=(kt == 0),
                                     stop=(not with_vbias and kt == KT - 1))
                if with_vbias:
                    nc.tensor.matmul(psv, lhsT=ones1_sb, rhs=vbias_sb,
                                     start=False, stop=True)
                nc.vector.tensor_copy(vv[:, m, :, 0:64],
                                      psv.rearrange("p (h d) -> p h d", h=HPC))

        def emit_scores(h, weave=()):
            """All 8 key-tiles of head h: score matmuls + exp. Up to two
            backlog emitters (AV matmuls / Z-chains) are woven in after each
            key-tile so the PE fills the exp-paced slack."""
            wq = list(weave)
            for m in range(MT):
                st = stp.tile([128, N], F32, tag="st", name=f"st{h}_{m}")
                for a, b in _score_ranges(m, q1, m1s):
                    nc.tensor.matmul(st[:, a:b],
                                     lhsT=kh[h][:, m * 128:(m + 1) * 128],
                                     rhs=qh[h][:, a:b],
                                     start=True, stop=True,
                                     skip_group_check=True)
                e = ep.tile([128, N], BF16, tag="e", name=f"e{h}_{m}")
                ea, eb = _exp_range(m, q1, m1s)
                nc.scalar.activation(e[:, ea:eb], st[:, ea:eb], AF.Exp)
                etiles[(h, m)] = e
                for _ in range(2):
                    if wq:
                        wq.pop(0)()
            for f in wq:
                f()

        def av_chunks(h, up):
            """Per-key-tile emitters for head h's AV accumulation. m=2
            (always full-width) goes first so every PSUM element is
            initialized by a start-group matmul; the partial-width tiles
            then accumulate per-element. The final emitter computes 1/Z
            (EPS is negligible: Z >= exp(s_ii); bf16 is a 0.4% common-mode
            scale on one head-query)."""
            order = [2] + [m for m in range(MT) if m != 2]

            def mk(i, m):
                def f():
                    if i == 0:
                        ups[h] = up.tile([65, N], F32, name=f"u{h}",
                                         tag="st" if up is stp else "u")
                    e = etiles.pop((h, m))
                    rs = _score_ranges(m, q1, m1s)
                    for k, (a, b) in enumerate(rs):
                        nc.tensor.matmul(
                            ups[h][:, a:b],
                            lhsT=V_sb[:, m, h * 65:(h + 1) * 65],
                            rhs=e[:, a:b],
                            start=(i == 0),
                            stop=(i == len(order) - 1 and k == len(rs) - 1),
                            skip_group_check=True)
                return f

            def zf():
                # recip (DVE) and the eager U->SBUF copy (ACT for the last
                # pair, DVE otherwise) are the only ups readers, so the
                # ups ring slot recycles ~2.4us sooner than a PSUM-side
                # normalize would allow.
                zrec[h] = zp.tile([1, N], BF16, tag="z", name=f"z{h}")
                with nc.allow_low_precision(reason="1/Z common-mode scale"):
                    nc.vector.reciprocal(zrec[h], ups[h][64:65, :])
                ust[h] = zp.tile([64, N], BF16, tag="ust", name=f"ust{h}")
                if h >= 4:
                    nc.scalar.activation(ust[h], ups[h][0:64, :], AF.Copy)
                else:
                    nc.vector.tensor_copy(ust[h], ups[h][0:64, :])

            return [mk(i, m) for i, m in enumerate(order)] + [zf]

        def emit_av(h, up):
            for f in av_chunks(h, up):
                f()

        def emit_zchain(h, zbs_eng=None):
            """Per-head normalize: Ab = Ust * broadcast(1/Z). Heads 0-3
            broadcast 1/Z across partitions via a DRAM bounce (no PSUM slot,
            no PE work; latency hides behind phase 2). The last pair is
            latency-critical and uses a PE ones-matmul into an stp slot."""
            tt, base = h // 2, 64 * (h % 2)
            if h < 4:
                nc.sync.dma_start(out=zd[h:h + 1, :], in_=zrec[h])
                zsrc = zd[h:h + 1, :]
                bsrc = bass.AP(tensor=zsrc.tensor, offset=zsrc.offset,
                               ap=[[0, 64]] + list(zsrc.ap[1:]))
                zbs = zp.tile([64, N], BF16, tag="zbs", name=f"zbs{h}")
                nc.sync.dma_start(out=zbs, in_=bsrc)
                nc.vector.tensor_mul(Ab[base:base + 64, tt, :],
                                     ust[h], zbs)
                return
            zbh = stp.tile([64, N], F32, tag="st", name=f"zbh{h}")
            for n in range(NT):
                sl = slice(n * 512, (n + 1) * 512)
                nc.tensor.matmul(zbh[:, sl], lhsT=ones64_sb,
                                 rhs=zrec[h][:, sl], start=True, stop=True)
            if h == HPC - 1:
                # split so the first proj tt2 matmuls start half a mul early
                for n in range(NT):
                    sl = slice(n * 512, (n + 1) * 512)
                    nc.vector.tensor_mul(Ab[base:base + 64, tt, sl],
                                         ust[h][:, sl], zbh[:, sl])
            else:
                nc.vector.tensor_mul(Ab[base:base + 64, tt, :],
                                     ust[h], zbh)

        # ---- interleaved schedule --------------------------------------
        with tc.tile_pool(name="ps1", bufs=1, space="PSUM") as ps1:
            emit_grp(ps1, 0, act_copies=True)
            emit_scores(0)
            emit_grp(ps1, 1)
            emit_scores(1)
            emit_scores(2)
            emit_grp(ps1, 2)
            emit_scores(3)
            emit_vgen(ps1)

        op = ctx.enter_context(tc.tile_pool(name="op", bufs=4))
        pst = {}

        def emit_proj_mm(pool, co, kts, start, stop):
            if co not in pst:
                pst[co] = pool.tile([128, N], F32, tag=pool is stp and "st"
                                    or "pj", name=f"pjps{co}")
            for kt in kts:
                for n in range(NT):
                    sl = slice(n * 512, (n + 1) * 512)
                    nc.tensor.matmul(
                        pst[co][:, sl],
                        lhsT=projT_sb[:, kt, co * 128:(co + 1) * 128],
                        rhs=Ab[:, kt, sl],
                        start=(start and kt == kts[0]),
                        stop=(stop and kt == kts[-1]),
                        skip_group_check=True)

        def emit_out(co, halves=1):
            ps = pst[co]
            ot = op.tile([128, N], BF16, tag="o", name=f"ot{co}")
            for i in range(halves):
                sl = slice(i * (N // halves), (i + 1) * (N // halves))
                eng = (nc.vector, nc.scalar)[(co + i) % 2]
                if eng is nc.vector:
                    nc.vector.tensor_scalar(
                        out=ot[:, sl], in0=ps[:, sl],
                        scalar1=pbias_sb[:, co:co + 1],
                        scalar2=None, op0=ALU.add)
                else:
                    nc.scalar.activation(ot[:, sl], ps[:, sl], AF.Identity,
                                         bias=pbias_sb[:, co:co + 1],
                                         scale=1.0)
                nc.sync.dma_start(out=t["outT"].ap()[:, co, sl],
                                  in_=ot[:, sl])

        with tc.tile_pool(name="up", bufs=2, space="PSUM") as up:
            backlog = (av_chunks(0, up) + [lambda: emit_zchain(0)]
                       + av_chunks(1, up) + [lambda: emit_zchain(1)])
            emit_scores(4, weave=backlog)
            backlog = (av_chunks(2, up) + [lambda: emit_zchain(2)]
                       + av_chunks(3, up) + [lambda: emit_zchain(3)]
                       + av_chunks(4, up)
                       + [lambda: emit_zchain(4, zbs_eng=nc.scalar)])
            emit_scores(5, weave=backlog)
            # Head 5's U accumulates in an stp-ring slot (emitted after all
            # h5 score tiles, so the ring stays deadlock-free). The up pool
            # then closes once head 4's readers finish, releasing banks for
            # the pj pool ~4us earlier.
            for f in av_chunks(5, stp):
                f()
            emit_zchain(5, zbs_eng=nc.scalar)
            emit_proj_mm(stp, 0, [0, 1], start=True, stop=False)
        # `up` closes once recip5/Ust5 are done, freeing banks for pj while
        # the mul5 chain drains.
        with tc.tile_pool(name="pj", bufs=2, space="PSUM") as pj:
            for co in (1, 2):
                emit_proj_mm(pj, co, [0, 1], start=True, stop=False)
            emit_proj_mm(stp, 0, [2], start=False, stop=True)
            emit_out(0)
            for co in (1, 2):
                emit_proj_mm(pj, co, [2], start=False, stop=True)
                emit_out(co)
            for co in range(3, CO):
                emit_proj_mm(pj, co, [0, 1, 2], start=True, stop=True)
                emit_out(co, halves=(2 if co == CO - 1 else 1))


_NC_CACHE = {}


def build_nc(reps: int = 1, with_vbias: bool = False, loop: int = 0,
             q1: int = N, m1s: int = MT):
    key = (reps, with_vbias, loop, q1, m1s)
    if key in _NC_CACHE:
        return _NC_CACHE[key]
    nc = bacc.Bacc("TRN2", target_bir_lowering=False, debug=False,
                   num_devices=N_CORES)
    t = {
        "xT": nc.dram_tensor("xT", [128, KT, N], BF16, kind="ExternalInput"),
        "wqkT": nc.dram_tensor("wqkT", [128, KT, C], BF16,
                               kind="ExternalInput"),
        "qkb": nc.dram_tensor("qkb", [128, KT], F32, kind="ExternalInput"),
        "wvT": nc.dram_tensor("wvT", [128, KT, HPC * HD], BF16,
                              kind="ExternalInput"),
        "vbias": nc.dram_tensor("vbias", [1, HPC * HD], BF16,
                                kind="ExternalInput"),
        "mq": nc.dram_tensor("mq", [2, N], BF16, kind="ExternalInput"),
        "mk": nc.dram_tensor("mk", [2, N], BF16, kind="ExternalInput"),
        "projT": nc.dram_tensor("projT", [128, 3, C], BF16,
                                kind="ExternalInput"),
        "pbias": nc.dram_tensor("pbias", [128, CO], F32,
                                kind="ExternalInput"),
        "outT": nc.dram_tensor("outT", [128, CO, N], BF16,
                               kind="ExternalOutput"),
    }
    with tile.TileContext(nc) as tc:
        if loop:
            with tc.For_i(0, loop, 1):
                _body(nc, tc, t, with_vbias, q1, m1s, warmup=False)
        else:
            for _ in range(reps):
                _body(nc, tc, t, with_vbias, q1, m1s)
    nc.compile()
    _NC_CACHE[key] = nc
    return nc


def _is_onehot(decision: np.ndarray) -> bool:
    vals_ok = np.all((decision == 0.0) | (decision == 1.0))
    return bool(vals_ok and np.all(decision.sum(-1) == 1.0))


def skip_params(decision, S):
    """Conservative shared skip bounds + per-batch token permutations.

    Token order per batch: [template | group2 | group1]. q1 = first query
    column that is group1 in EVERY batch's layout; key tiles >= m1s are
    group1 in every batch. Falls back to dense when the bounds give no
    safely skippable region.
    """
    tpl = N - S
    perms = []
    g2s = []
    for b in range(B):
        g2idx = np.where(decision[b][:, 1] == 1.0)[0]
        g1idx = np.where(decision[b][:, 1] == 0.0)[0]
        perms.append(np.concatenate(
            [np.arange(tpl), tpl + g2idx, tpl + g1idx]))
        g2s.append(len(g2idx))
    q1 = tpl + max(g2s)
    m1s = -(-q1 // 128)          # ceil
    if tpl != 256 or q1 >= N or m1s < 3 or q1 <= 512:
        return N, MT, perms      # dense fallback
    return q1, m1s, perms


def make_in_maps(x, decision, qkv_w, qkv_b, proj_w, proj_b, S, perms):
    in_maps = []
    xT_cache = {}
    for core in range(N_CORES):
        b, hg = core // 2, core % 2
        perm = perms[b]
        if b not in xT_cache:
            xT = np.ascontiguousarray(x[b].T[:, perm])  # [C, N] permuted
            xT_cache[b] = np.ascontiguousarray(
                xT.reshape(KT, 128, N).transpose(1, 0, 2)).astype(BFNP)
        qs = slice(hg * 384, hg * 384 + 384)
        ks = slice(C + hg * 384, C + hg * 384 + 384)
        vs = slice(2 * C + hg * 384, 2 * C + hg * 384 + 384)
        Wqk = np.concatenate([qkv_w[qs] * SCALE, qkv_w[ks]], axis=0)
        bqk = np.concatenate([qkv_b[qs] * SCALE, qkv_b[ks]])
        # j-interleave the 128-row output blocks: [q0|k0|q1|k1|q2|k2] so
        # each head-pair group's weight columns are contiguous in wqkT.
        order = [0, 3, 1, 4, 2, 5]
        Wqk = np.concatenate([Wqk[j * 128:(j + 1) * 128] for j in order])
        bqk = np.concatenate([bqk[j * 128:(j + 1) * 128] for j in order])
        wqkT = np.ascontiguousarray(
            Wqk.T.reshape(KT, 128, C).transpose(1, 0, 2)).astype(BFNP)
        qkb = np.ascontiguousarray(bqk.reshape(KT, 128).T, dtype=np.float32)
        wvT = np.ascontiguousarray(
            qkv_w[vs].T.reshape(KT, 128, 384).transpose(1, 0, 2)).astype(BFNP)
        vbias = qkv_b[vs].reshape(1, 384).astype(BFNP)
        p0 = np.zeros(N, np.float32)
        p0[:N - S] = 1.0
        p1 = np.zeros(N, np.float32)
        p1[N - S:] = decision[b][:, 0]
        p0, p1 = p0[perm], p1[perm]
        mq = np.stack([p1, p0]).astype(BFNP)
        mk = np.stack([-BIG * p0, -BIG * p1]).astype(BFNP)
        projT = np.ascontiguousarray(
            proj_w[:, hg * 384:hg * 384 + 384].T
            .reshape(3, 128, C).transpose(1, 0, 2)).astype(BFNP)
        if hg == 0:
            pbias = np.ascontiguousarray(
                proj_b.reshape(CO, 128).T, dtype=np.float32)
        else:
            pbias = np.zeros((128, CO), np.float32)
        in_maps.append({
            "xT": xT_cache[b], "wqkT": wqkT, "qkb": qkb, "wvT": wvT,
            "vbias": vbias, "mq": mq, "mk": mk,
            "projT": projT, "pbias": pbias,
        })
    return in_maps


def _numpy_fallback(x, decision, qkv_w, qkv_b, proj_w, proj_b, S):
    """Direct port of the reference for non-one-hot policies."""
    EPS = 1e-6
    out = np.empty((B, N, C), np.float32)
    for b in range(B):
        p0 = np.zeros(N, np.float32)
        p0[:N - S] = 1.0
        p1 = np.zeros(N, np.float32)
        p1[N - S:] = decision[b][:, 0]
        p2 = np.zeros(N, np.float32)
        p2[N - S:] = decision[b][:, 1]
        qkv = x[b] @ qkv_w.T + qkv_b
        qkv = qkv.reshape(N, 3, H, HD).transpose(1, 2, 0, 3)
        q, k, v = qkv[0], qkv[1], qkv[2]
        s = p0 + p1 + p2
        ap = (np.outer(s, s) - np.outer(p0, p1) - np.outer(p1, p0))
        ap = ap + (1.0 - ap) * np.eye(N, dtype=np.float32)
        attn = np.einsum('hnd,hmd->hnm', q, k).astype(np.float32) * SCALE
        m = attn.max(-1, keepdims=True)
        e = np.exp(attn - m) * ap[None]
        p = (e + EPS / N) / (e.sum(-1, keepdims=True) + EPS)
        o = np.einsum('hnm,hmd->hnd', p, v)
        out[b] = o.transpose(1, 0, 2).reshape(N, C) @ proj_w.T + proj_b
    return out


def kernel(x, decision, qkv_w, qkv_b, proj_w, proj_b, search_feat_len):
    x = np.asarray(x, np.float32)
    decision = np.asarray(decision, np.float32)
    qkv_w = np.asarray(qkv_w, np.float32)
    qkv_b = np.asarray(qkv_b, np.float32)
    proj_w = np.asarray(proj_w, np.float32)
    proj_b = np.asarray(proj_b, np.float32)
    S = int(np.asarray(search_feat_len))

    if not _is_onehot(decision):
        return _numpy_fallback(x, decision, qkv_w, qkv_b, proj_w, proj_b, S)

    q1, m1s, perms = skip_params(decision, S)
    nc = build_nc(with_vbias=bool(np.any(qkv_b[2 * C:] != 0.0)),
                  q1=q1, m1s=m1s)
    in_maps = make_in_maps(x, decision, qkv_w, qkv_b, proj_w, proj_b, S,
                           perms)
    res = run_bass_kernel_spmd(nc, in_maps, core_ids=list(range(N_CORES)))

    out = np.empty((B, N, C), np.float32)
    for b in range(B):
        partial = (res.results[2 * b]["outT"].astype(np.float32)
                   + res.results[2 * b + 1]["outT"].astype(np.float32))
        out[b][perms[b]] = partial.transpose(1, 0, 2).reshape(C, N).T
    return out


# revision 15
# speedup vs baseline: 1.4007x; 1.0078x over previous
"""Trainium2 Bass kernel for policy-masked attention (sparse_attention), v3.

Shapes (hardcoded): x [4,1024,768], decision [4,768,2], qkv_w [2304,768],
qkv_b [2304], proj_w [768,768], proj_b [768], search_feat_len=768.

Sharding: 8 cores = 4 batches x 2 head-groups (6 heads each). Each core
computes its batch's q/k/v for its heads, the policy-masked softmax
(one-hot policy folded into the score matmul as 2 extra contraction rows
of -BIG * indicator outer products), attn @ v with a fused ones-column
producing the softmax denominator, and a partial output projection.
Host sums the two head-group partials per batch.

v3 design:
- all PE operands bf16 (same PE rate as fp32r, half DMA/SBUF).
- host sorts tokens [template | group2 | group1] per batch, so the two
  masked blocks (template-q x group1-k and group1-q x template-k) become
  contiguous; score/exp/AV skip the conservatively-safe common region
  (bounds shared across cores: q1 = tpl + max_b g2). The rank-2 -BIG mask
  rows still handle all boundary tiles exactly.
- interleaved emission: QK-gen j-groups and the first four heads' score
  streams alternate, so the scalar engine's exp stream starts ~6us in and
  runs gapless; AV matmuls for those heads run later against buffered e
  tiles. V-gen sits between; proj tt0/tt1 matmuls overlap the last
  Z-chain.
- softmax 1/Z: DVE reciprocal of the fused Z row, PE ones-matmul
  broadcast, one PSUM->SBUF stage (single-PSUM-operand rule), DVE muls.
- AV accumulation emits the full-width m=2 tile first so every PSUM
  element is has_written-initialized regardless of clear granularity.
"""
import numpy as np
import ml_dtypes

import concourse.bass as bass
import concourse.tile as tile
from concourse import bacc, mybir
from concourse.bass_utils import run_bass_kernel_spmd

F32 = mybir.dt.float32
BF16 = mybir.dt.bfloat16
AF = mybir.ActivationFunctionType
ALU = mybir.AluOpType
BFNP = ml_dtypes.bfloat16

B, N, C = 4, 1024, 768
H = 12
HD = 64
HPC = 6              # heads per core
KT = C // 128        # 6 contraction tiles
NT = N // 512        # 2 moving slices
MT = N // 128        # 8 key tiles
CO = C // 128        # 6 output-column tiles
SCALE = HD ** -0.5
BIG = 32768.0
N_CORES = 8


def _score_ranges(m, q1, m1s):
    """Query-column ranges to compute for key-tile m (512-col chunks)."""
    if m < 2 and q1 < N:          # template keys: skip group1 queries
        return [(0, 512), (512, q1)] if q1 > 512 else [(0, q1)]
    if m >= m1s:                  # group1 keys: skip template queries
        return [(256, 512), (512, 1024)]
    return [(0, 512), (512, 1024)]


def _exp_range(m, q1, m1s):
    if m < 2 and q1 < N:
        return (0, q1)
    if m >= m1s:
        return (256, N)
    return (0, N)


def _body(nc, tc, t, with_vbias, q1, m1s, warmup=True):
    import contextlib
    with contextlib.ExitStack() as ctx:
        consts = ctx.enter_context(tc.tile_pool(name="consts", bufs=1))
        headp = ctx.enter_context(tc.tile_pool(name="headp", bufs=1))

        xT_sb = consts.tile([128, KT, N], BF16)
        wqkT_sb = consts.tile([128, KT, C], BF16)
        wvT_sb = consts.tile([128, KT, HPC * HD], BF16)
        vbias_sb = consts.tile([1, HPC * HD], BF16)
        ones1_sb = consts.tile([1, 128], BF16)
        ones64_sb = consts.tile([1, 64], BF16)
        qkb_sb = consts.tile([128, KT], F32)
        projT_sb = consts.tile([128, 3, C], BF16)
        pbias_sb = consts.tile([128, CO], F32)

        # Per-head q/k tiles [66, N]: rows 0-63 head data, rows 64-65 the
        # rank-2 log-mask factors (k side: -BIG*p0,-BIG*p1; q side: p1,p0).
        qh = [headp.tile([66, N], BF16, name=f"qh{h}", tag=f"qh{h}")
              for h in range(HPC)]
        kh = [headp.tile([66, N], BF16, name=f"kh{h}", tag=f"kh{h}")
              for h in range(HPC)]

        # Every dma_start costs ~1.2us of SEQ time on the issuing engine, so
        # batch the inputs into few large DMAs and keep the scalar (ACT)
        # queue nearly empty -- its sequencer must be free for the exp
        # stream. wqkT is stored j-interleaved ([j0|j3|j1|j4|j2|j5]) so each
        # group's 256 weight columns are one strided DMA; x lands in three
        # ascending pieces so group 0's kt-outer matmuls start early.
        nc.scalar.dma_start(out=wqkT_sb[:, :, 0:256],
                            in_=t["wqkT"].ap()[:, :, 0:256])
        nc.scalar.dma_start(out=qh[0][64:66, :], in_=t["mq"].ap())
        nc.scalar.dma_start(out=qkb_sb, in_=t["qkb"].ap())
        if with_vbias:
            nc.scalar.dma_start(out=vbias_sb, in_=t["vbias"].ap())
        nc.sync.dma_start(out=xT_sb[:, 0, :], in_=t["xT"].ap()[:, 0, :])
        nc.sync.dma_start(out=kh[0][64:66, :], in_=t["mk"].ap())
        nc.sync.dma_start(out=xT_sb[:, 1:3, :], in_=t["xT"].ap()[:, 1:3, :])
        nc.sync.dma_start(out=xT_sb[:, 3:6, :], in_=t["xT"].ap()[:, 3:6, :])
        nc.sync.dma_start(out=wqkT_sb[:, :, 256:768],
                          in_=t["wqkT"].ap()[:, :, 256:768])
        nc.sync.dma_start(out=wvT_sb, in_=t["wvT"].ap())
        for h in range(1, HPC):
            nc.sync.dma_start(out=qh[h][64:66, :], in_=t["mq"].ap())
            nc.sync.dma_start(out=kh[h][64:66, :], in_=t["mk"].ap())
        nc.sync.dma_start(out=pbias_sb, in_=t["pbias"].ap())
        nc.sync.dma_start(out=projT_sb, in_=t["projT"].ap())
        nc.vector.memset(ones1_sb, 1.0)
        nc.vector.memset(ones64_sb, 1.0)

        # V in token-major layout with a fused ones column: [128, MT, 6*65]
        V_sb = consts.tile([128, MT, HPC * 65], BF16)
        vv = V_sb.rearrange("p m (h e) -> p m h e", h=HPC)
        nc.vector.memset(vv[:, :, :, 64:65], 1.0)

        # Pre-load the Exp activation table off the critical path.
        warm = consts.tile([1, 1], F32)
        nc.scalar.activation(warm, qkb_sb[0:1, 0:1], AF.Exp)


        abp = ctx.enter_context(tc.tile_pool(name="abp", bufs=1))
        Ab = abp.tile([128, 3, N], BF16)

        stp = ctx.enter_context(tc.tile_pool(name="stp", bufs=2, space="PSUM"))

        # Warm up the PE p-state/HAM during the initial DMA window: ~4.5us
        # of dummy matmuls on memset constants so the real QK-gen matmuls
        # run at full clock. Serialized WAW on one PSUM tile keeps them
        # back-to-back; one DVE read is the ring consumer.
        if warmup:
            wrhs = consts.tile([1, 512], BF16)
            nc.vector.memset(wrhs, 0.5)
            wps = stp.tile([64, 512], F32, tag="st", name="wps")
            for _ in range(9):
                nc.tensor.matmul(wps, lhsT=ones64_sb, rhs=wrhs,
                                 start=True, stop=True)
            wsink = consts.tile([1, 1], F32)
            nc.vector.tensor_copy(wsink, wps[0:1, 0:1])

        ep = ctx.enter_context(tc.tile_pool(name="ep", bufs=48))
        zp = ctx.enter_context(tc.tile_pool(name="zp", bufs=4))
        zdram = ctx.enter_context(tc.tile_pool(name="zdram", bufs=1,
                                               space="DRAM"))
        zd = zdram.tile([4, N], BF16)

        ups, zrec, ust, etiles = {}, {}, {}, {}

        # ---- emission helpers ------------------------------------------
        def emit_grp(ps1, g, act_copies=False):
            """QK-gen for j-group {g, g+3} -> heads 2g, 2g+1. kt-outer and
            n-inner: all four [128,512] PSUM tiles accumulate together, so
            the matmuls pace with the x/w DMA chunk stream. wqkT column
            block 2g holds the q rows, 2g+1 the k rows (j-interleaved)."""
            for i, (p, n) in enumerate(
                    (p, n) for p in (2 * g, 2 * g + 1) for n in range(NT)):
                ps = ps1.tile([128, 512], F32, bufs=1,
                              tag=f"g{'qk'[p % 2]}{n}", name=f"g{g}_{p}_{n}")
                for kt in range(KT):
                    nc.tensor.matmul(
                        ps,
                        lhsT=wqkT_sb[:, kt, p * 128:(p + 1) * 128],
                        rhs=xT_sb[:, kt, n * 512:(n + 1) * 512],
                        start=(kt == 0), stop=(kt == KT - 1))
                tiles = qh if p % 2 == 0 else kh
                sl = slice(n * 512, (n + 1) * 512)
                for half in range(2):
                    h = 2 * g + half
                    dst = tiles[h][0:64, sl]
                    srcap = ps[half * 64:(half + 1) * 64, :]
                    bias = qkb_sb[half * 64:(half + 1) * 64, p:p + 1]
                    if act_copies and (i + half) % 2 == 1:
                        nc.scalar.activation(dst, srcap, AF.Identity,
                                             bias=bias, scale=1.0)
                    else:
                        nc.vector.tensor_scalar(out=dst, in0=srcap,
                                                scalar1=bias, scalar2=None,
                                                op0=ALU.add)

        def emit_vgen(ps1):
            for m in range(MT):
                psv = ps1.tile([128, HPC * HD], F32,
                               tag=f"gq{m % 2}", bufs=1)
                for kt in range(KT):
                    nc.tensor.matmul(psv,
                                     lhsT=xT_sb[:, kt, m * 128:(m + 1) * 128],
                                     rhs=wvT_sb[:, kt, :],
                                     start=(kt == 0),
                                     stop=(not with_vbias and kt == KT - 1))
                if with_vbias:
                    nc.tensor.matmul(psv, lhsT=ones1_sb, rhs=vbias_sb,
                                     start=False, stop=True)
                nc.vector.tensor_copy(vv[:, m, :, 0:64],
                                      psv.rearrange("p (h d) -> p h d", h=HPC))

        def emit_scores(h, weave=(), pops=2):
            """All 8 key-tiles of head h: score matmuls + exp. Up to two
            backlog emitters (AV matmuls / Z-chains) are woven in after each
            key-tile so the PE fills the exp-paced slack."""
            wq = list(weave)
            for m in range(MT):
                st = stp.tile([128, N], F32, tag="st", name=f"st{h}_{m}")
                for a, b in _score_ranges(m, q1, m1s):
                    nc.tensor.matmul(st[:, a:b],
                                     lhsT=kh[h][:, m * 128:(m + 1) * 128],
                                     rhs=qh[h][:, a:b],
                                     start=True, stop=True,
                                     skip_group_check=True)
                e = ep.tile([128, N], BF16, tag="e", name=f"e{h}_{m}")
                ea, eb = _exp_range(m, q1, m1s)
                nc.scalar.activation(e[:, ea:eb], st[:, ea:eb], AF.Exp)
                etiles[(h, m)] = e
                for _ in range(pops):
                    if wq:
                        wq.pop(0)()
            for f in wq:
                f()

        def av_chunks(h, up):
            """Per-key-tile emitters for head h's AV accumulation. m=2
            (always full-width) goes first so every PSUM element is
            initialized by a start-group matmul; the partial-width tiles
            then accumulate per-element. The final emitter computes 1/Z
            (EPS is negligible: Z >= exp(s_ii); bf16 is a 0.4% common-mode
            scale on one head-query)."""
            order = [2] + [m for m in range(MT) if m != 2]

            def mk(i, m):
                def f():
                    if i == 0:
                        ups[h] = up.tile([65, N], F32, name=f"u{h}",
                                         tag="st" if up is stp else "u")
                    e = etiles.pop((h, m))
                    rs = _score_ranges(m, q1, m1s)
                    for k, (a, b) in enumerate(rs):
                        nc.tensor.matmul(
                            ups[h][:, a:b],
                            lhsT=V_sb[:, m, h * 65:(h + 1) * 65],
                            rhs=e[:, a:b],
                            start=(i == 0),
                            stop=(i == len(order) - 1 and k == len(rs) - 1),
                            skip_group_check=True)
                return f

            def zf():
                # recip (DVE) and the eager U->SBUF copy (ACT for the last
                # pair, DVE otherwise) are the only ups readers, so the
                # ups ring slot recycles ~2.4us sooner than a PSUM-side
                # normalize would allow.
                zrec[h] = zp.tile([1, N], BF16, tag="z", name=f"z{h}")
                with nc.allow_low_precision(reason="1/Z common-mode scale"):
                    nc.vector.reciprocal(zrec[h], ups[h][64:65, :])
                ust[h] = zp.tile([64, N], BF16, tag="ust", name=f"ust{h}")
                if h >= 4:
                    nc.scalar.activation(ust[h], ups[h][0:64, :], AF.Copy)
                else:
                    nc.vector.tensor_copy(ust[h], ups[h][0:64, :])

            return [mk(i, m) for i, m in enumerate(order)] + [zf]

        def emit_av(h, up):
            for f in av_chunks(h, up):
                f()

        def emit_zchain(h, zbs_eng=None):
            """Per-head normalize: Ab = Ust * broadcast(1/Z). Heads 0-3
            broadcast 1/Z across partitions via a DRAM bounce (no PSUM slot,
            no PE work; latency hides behind phase 2). The last pair is
            latency-critical and uses a PE ones-matmul into an stp slot."""
            tt, base = h // 2, 64 * (h % 2)
            if h < 4:
                nc.sync.dma_start(out=zd[h:h + 1, :], in_=zrec[h])
                zsrc = zd[h:h + 1, :]
                bsrc = bass.AP(tensor=zsrc.tensor, offset=zsrc.offset,
                               ap=[[0, 64]] + list(zsrc.ap[1:]))
                zbs = zp.tile([64, N], BF16, tag="zbs", name=f"zbs{h}")
                nc.sync.dma_start(out=zbs, in_=bsrc)
                nc.vector.tensor_mul(Ab[base:base + 64, tt, :],
                                     ust[h], zbs)
                return
            zbh = stp.tile([64, N], F32, tag="st", name=f"zbh{h}")
            for n in range(NT):
                sl = slice(n * 512, (n + 1) * 512)
                nc.tensor.matmul(zbh[:, sl], lhsT=ones64_sb,
                                 rhs=zrec[h][:, sl], start=True, stop=True)
            if h == HPC - 1:
                # split so the first proj tt2 matmuls start half a mul early
                for n in range(NT):
                    sl = slice(n * 512, (n + 1) * 512)
                    nc.vector.tensor_mul(Ab[base:base + 64, tt, sl],
                                         ust[h][:, sl], zbh[:, sl])
            else:
                nc.vector.tensor_mul(Ab[base:base + 64, tt, :],
                                     ust[h], zbh)

        # ---- interleaved schedule --------------------------------------
        with tc.tile_pool(name="ps1", bufs=1, space="PSUM") as ps1:
            emit_grp(ps1, 0, act_copies=True)
            emit_scores(0)
            emit_grp(ps1, 1)
            emit_scores(1)
            emit_scores(2)
            emit_grp(ps1, 2)
            emit_scores(3)
            emit_vgen(ps1)

        op = ctx.enter_context(tc.tile_pool(name="op", bufs=4))
        pst = {}

        def emit_proj_mm(pool, co, kts, start, stop):
            if co not in pst:
                pst[co] = pool.tile([128, N], F32, tag=pool is stp and "st"
                                    or "pj", name=f"pjps{co}")
            for kt in kts:
                for n in range(NT):
                    sl = slice(n * 512, (n + 1) * 512)
                    nc.tensor.matmul(
                        pst[co][:, sl],
                        lhsT=projT_sb[:, kt, co * 128:(co + 1) * 128],
                        rhs=Ab[:, kt, sl],
                        start=(start and kt == kts[0]),
                        stop=(stop and kt == kts[-1]),
                        skip_group_check=True)

        def emit_out(co, halves=1):
            ps = pst[co]
            ot = op.tile([128, N], BF16, tag="o", name=f"ot{co}")
            for i in range(halves):
                sl = slice(i * (N // halves), (i + 1) * (N // halves))
                eng = (nc.vector, nc.scalar)[(co + i) % 2]
                if eng is nc.vector:
                    nc.vector.tensor_scalar(
                        out=ot[:, sl], in0=ps[:, sl],
                        scalar1=pbias_sb[:, co:co + 1],
                        scalar2=None, op0=ALU.add)
                else:
                    nc.scalar.activation(ot[:, sl], ps[:, sl], AF.Identity,
                                         bias=pbias_sb[:, co:co + 1],
                                         scale=1.0)
                nc.sync.dma_start(out=t["outT"].ap()[:, co, sl],
                                  in_=ot[:, sl])

        with tc.tile_pool(name="up", bufs=2, space="PSUM") as up:
            backlog = (av_chunks(0, up) + [lambda: emit_zchain(0)]
                       + av_chunks(1, up) + [lambda: emit_zchain(1)])
            emit_scores(4, weave=backlog, pops=3)
            backlog = (av_chunks(2, up) + [lambda: emit_zchain(2)]
                       + av_chunks(3, up) + [lambda: emit_zchain(3)]
                       + av_chunks(4, up)
                       + [lambda: emit_zchain(4, zbs_eng=nc.scalar)])
            emit_scores(5, weave=backlog, pops=4)
            # Head 5's U accumulates in an stp-ring slot (emitted after all
            # h5 score tiles, so the ring stays deadlock-free). The up pool
            # then closes once head 4's readers finish, releasing banks for
            # the pj pool ~4us earlier.
            for f in av_chunks(5, stp):
                f()
            emit_zchain(5, zbs_eng=nc.scalar)
            emit_proj_mm(stp, 0, [0, 1], start=True, stop=False)
        # `up` closes once recip5/Ust5 are done, freeing banks for pj while
        # the mul5 chain drains.
        with tc.tile_pool(name="pj", bufs=2, space="PSUM") as pj:
            for co in (1, 2):
                emit_proj_mm(pj, co, [0, 1], start=True, stop=False)
            emit_proj_mm(stp, 0, [2], start=False, stop=True)
            emit_out(0)
            for co in (1, 2):
                emit_proj_mm(pj, co, [2], start=False, stop=True)
                emit_out(co)
            for co in range(3, CO):
                emit_proj_mm(pj, co, [0, 1, 2], start=True, stop=True)
                emit_out(co, halves=(2 if co == CO - 1 else 1))


_NC_CACHE = {}


def build_nc(reps: int = 1, with_vbias: bool = False, loop: int = 0,
             q1: int = N, m1s: int = MT):
    key = (reps, with_vbias, loop, q1, m1s)
    if key in _NC_CACHE:
        return _NC_CACHE[key]
    nc = bacc.Bacc("TRN2", target_bir_lowering=False, debug=False,
                   num_devices=N_CORES)
    t = {
        "xT": nc.dram_tensor("xT", [128, KT, N], BF16, kind="ExternalInput"),
        "wqkT": nc.dram_tensor("wqkT", [128, KT, C], BF16,
                               kind="ExternalInput"),
        "qkb": nc.dram_tensor("qkb", [128, KT], F32, kind="ExternalInput"),
        "wvT": nc.dram_tensor("wvT", [128, KT, HPC * HD], BF16,
                              kind="ExternalInput"),
        "vbias": nc.dram_tensor("vbias", [1, HPC * HD], BF16,
                                kind="ExternalInput"),
        "mq": nc.dram_tensor("mq", [2, N], BF16, kind="ExternalInput"),
        "mk": nc.dram_tensor("mk", [2, N], BF16, kind="ExternalInput"),
        "projT": nc.dram_tensor("projT", [128, 3, C], BF16,
                                kind="ExternalInput"),
        "pbias": nc.dram_tensor("pbias", [128, CO], F32,
                                kind="ExternalInput"),
        "outT": nc.dram_tensor("outT", [128, CO, N], BF16,
                               kind="ExternalOutput"),
    }
    with tile.TileContext(nc) as tc:
        if loop:
            with tc.For_i(0, loop, 1):
                _body(nc, tc, t, with_vbias, q1, m1s, warmup=False)
        else:
            for _ in range(reps):
                _body(nc, tc, t, with_vbias, q1, m1s)
    nc.compile()
    _NC_CACHE[key] = nc
    return nc


def _is_onehot(decision: np.ndarray) -> bool:
    vals_ok = np.all((decision == 0.0) | (decision == 1.0))
    return bool(vals_ok and np.all(decision.sum(-1) == 1.0))


def skip_params(decision, S):
    """Conservative shared skip bounds + per-batch token permutations.

    Token order per batch: [template | group2 | group1]. q1 = first query
    column that is group1 in EVERY batch's layout; key tiles >= m1s are
    group1 in every batch. Falls back to dense when the bounds give no
    safely skippable region.
    """
    tpl = N - S
    perms = []
    g2s = []
    for b in range(B):
        g2idx = np.where(decision[b][:, 1] == 1.0)[0]
        g1idx = np.where(decision[b][:, 1] == 0.0)[0]
        perms.append(np.concatenate(
            [np.arange(tpl), tpl + g2idx, tpl + g1idx]))
        g2s.append(len(g2idx))
    q1 = tpl + max(g2s)
    m1s = -(-q1 // 128)          # ceil
    if tpl != 256 or q1 >= N or m1s < 3 or q1 <= 512:
        return N, MT, perms      # dense fallback
    return q1, m1s, perms


def make_in_maps(x, decision, qkv_w, qkv_b, proj_w, proj_b, S, perms):
    in_maps = []
    xT_cache = {}
    for core in range(N_CORES):
        b, hg = core // 2, core % 2
        perm = perms[b]
        if b not in xT_cache:
            xT = np.ascontiguousarray(x[b].T[:, perm])  # [C, N] permuted
            xT_cache[b] = np.ascontiguousarray(
                xT.reshape(KT, 128, N).transpose(1, 0, 2)).astype(BFNP)
        qs = slice(hg * 384, hg * 384 + 384)
        ks = slice(C + hg * 384, C + hg * 384 + 384)
        vs = slice(2 * C + hg * 384, 2 * C + hg * 384 + 384)
        Wqk = np.concatenate([qkv_w[qs] * SCALE, qkv_w[ks]], axis=0)
        bqk = np.concatenate([qkv_b[qs] * SCALE, qkv_b[ks]])
        # j-interleave the 128-row output blocks: [q0|k0|q1|k1|q2|k2] so
        # each head-pair group's weight columns are contiguous in wqkT.
        order = [0, 3, 1, 4, 2, 5]
        Wqk = np.concatenate([Wqk[j * 128:(j + 1) * 128] for j in order])
        bqk = np.concatenate([bqk[j * 128:(j + 1) * 128] for j in order])
        wqkT = np.ascontiguousarray(
            Wqk.T.reshape(KT, 128, C).transpose(1, 0, 2)).astype(BFNP)
        qkb = np.ascontiguousarray(bqk.reshape(KT, 128).T, dtype=np.float32)
        wvT = np.ascontiguousarray(
            qkv_w[vs].T.reshape(KT, 128, 384).transpose(1, 0, 2)).astype(BFNP)
        vbias = qkv_b[vs].reshape(1, 384).astype(BFNP)
        p0 = np.zeros(N, np.float32)
        p0[:N - S] = 1.0
        p1 = np.zeros(N, np.float32)
        p1[N - S:] = decision[b][:, 0]
        p0, p1 = p0[perm], p1[perm]
        mq = np.stack([p1, p0]).astype(BFNP)
        mk = np.stack([-BIG * p0, -BIG * p1]).astype(BFNP)
        projT = np.ascontiguousarray(
            proj_w[:, hg * 384:hg * 384 + 384].T
            .reshape(3, 128, C).transpose(1, 0, 2)).astype(BFNP)
        if hg == 0:
            pbias = np.ascontiguousarray(
                proj_b.reshape(CO, 128).T, dtype=np.float32)
        else:
            pbias = np.zeros((128, CO), np.float32)
        in_maps.append({
            "xT": xT_cache[b], "wqkT": wqkT, "qkb": qkb, "wvT": wvT,
            "vbias": vbias, "mq": mq, "mk": mk,
            "projT": projT, "pbias": pbias,
        })
    return in_maps


def _numpy_fallback(x, decision, qkv_w, qkv_b, proj_w, proj_b, S):
    """Direct port of the reference for non-one-hot policies."""
    EPS = 1e-6
    out = np.empty((B, N, C), np.float32)
    for b in range(B):
        p0 = np.zeros(N, np.float32)
        p0[:N - S] = 1.0
        p1 = np.zeros(N, np.float32)
        p1[N - S:] = decision[b][:, 0]
        p2 = np.zeros(N, np.float32)
        p2[N - S:] = decision[b][:, 1]
        qkv = x[b] @ qkv_w.T + qkv_b
        qkv = qkv.reshape(N, 3, H, HD).transpose(1, 2, 0, 3)
        q, k, v = qkv[0], qkv[1], qkv[2]
        s = p0 + p1 + p2
        ap = (np.outer(s, s) - np.outer(p0, p1) - np.outer(p1, p0))
        ap = ap + (1.0 - ap) * np.eye(N, dtype=np.float32)
        attn = np.einsum('hnd,hmd->hnm', q, k).astype(np.float32) * SCALE
        m = attn.max(-1, keepdims=True)
        e = np.exp(attn - m) * ap[None]
        p = (e + EPS / N) / (e.sum(-1, keepdims=True) + EPS)
        o = np.einsum('hnm,hmd->hnd', p, v)
        out[b] = o.transpose(1, 0, 2).reshape(N, C) @ proj_w.T + proj_b
    return out


def kernel(x, decision, qkv_w, qkv_b, proj_w, proj_b, search_feat_len):
    x = np.asarray(x, np.float32)
    decision = np.asarray(decision, np.float32)
    qkv_w = np.asarray(qkv_w, np.float32)
    qkv_b = np.asarray(qkv_b, np.float32)
    proj_w = np.asarray(proj_w, np.float32)
    proj_b = np.asarray(proj_b, np.float32)
    S = int(np.asarray(search_feat_len))

    if not _is_onehot(decision):
        return _numpy_fallback(x, decision, qkv_w, qkv_b, proj_w, proj_b, S)

    q1, m1s, perms = skip_params(decision, S)
    nc = build_nc(with_vbias=bool(np.any(qkv_b[2 * C:] != 0.0)),
                  q1=q1, m1s=m1s)
    in_maps = make_in_maps(x, decision, qkv_w, qkv_b, proj_w, proj_b, S,
                           perms)
    res = run_bass_kernel_spmd(nc, in_maps, core_ids=list(range(N_CORES)))

    out = np.empty((B, N, C), np.float32)
    for b in range(B):
        partial = (res.results[2 * b]["outT"].astype(np.float32)
                   + res.results[2 * b + 1]["outT"].astype(np.float32))
        out[b][perms[b]] = partial.transpose(1, 0, 2).reshape(C, N).T
    return out
